# revision 38
# baseline (speedup 1.0000x reference)
"""Multi-head attention forward (B=4, N=1024, D=768, H=12, dh=64) on 8 TRN2 cores.

Sharding: (batch, head-group) — core c handles batch b = c//2 and heads
hs..hs+5 where hs = (c%2)*6.  Each core computes its 6 heads' contribution
to out[b] = attn(x[b]) @ W_out_rows(for its heads); host sums the two
partials per batch and adds the bias (the "all-reduce after final linear").

Per-core dataflow (all contraction dims on SBUF partitions), fp16 wire
dtype (host casts; fp32 PSUM accumulation; ~1e-3 end-to-end rel err):
  qkT  [768,1024] = w_qk^T @ x^T          (d-major q,k — feeds scores;
                                           w_qk cols pair-packed
                                           [q_p0|k_p0|q_p1|k_p1|q_p2|k_p2])
  v    [1024,390] = x @ w_v (+ ones col)  (n-major v — feeds AV^T)
  S^T  [1024,1024]/head = k_h @ q_h^T     (keys on partitions, 2 heads
                                           row-packed in the PE array,
                                           both heads' scores in one
                                           2-bank PSUM tile so each
                                           (i, pair) step is ONE 1024-wide
                                           exp — ACT is the weave pacer)
  P^T  = exp(S^T * scale)                 (no max-sub: scores ~ N(0,1))
  oT   [65,1024]/head = [v_h|1]^T @ P^T   (row 64 = softmax denominators)
  attT = oT[0:64] * (1/denom)             (K=1 matmul broadcasts 1/denom
                                           across partitions, DVE mult)
  out  [1024,768] = attT^T @ w_o          (partial; host all-reduce)

The weave: filler work (projections for later pairs, v blocks, finished
output rows) is interleaved into each attention unit's emission stream so
the tensor engine's exp-latency bubbles are filled, balanced against the
unit's ~8.3us of ACT exp work. The last unit ends with a pipelined tail:
row partials first, then the normalize chain split even/odd so the final
j=2 matmuls read the odd-head tile directly (no SBUF->SBUF partition-shift
DMA on the critical path), one output DMA per finished half-row.
"""
import os
import sys

sys.path.insert(0, "/opt/trn_rl_repo")

# The kernel needs the axon-tunneled TRN2 PJRT backend; a JAX_PLATFORMS=cpu
# pin (common for reference-side jax) would hide the NeuronCores.
if os.environ.get("JAX_PLATFORMS", "").strip() == "cpu":
    del os.environ["JAX_PLATFORMS"]

import numpy as np
import concourse.bass as bass
import concourse.bacc as bacc
import concourse.tile as tile
from concourse import mybir
from concourse.bass_utils import run_bass_kernel_spmd
from contextlib import ExitStack

F32 = mybir.dt.float32
F32R = mybir.dt.float32r
F16 = mybir.dt.float16

DIM = 768
N = 1024
HEADS_PER_CORE = 6
DH = 64
SCALE = DH ** -0.5
NCORES = 8

# "f16"  = fp16 pipeline (1 cyc/row PE at any width, half DMA traffic;
#          ~1e-3 end-to-end rel err)
# "f32r" = TF32-like matmul mode (~4e-4 end-to-end rel err)
MODE = os.environ.get("ATTN_MM_DTYPE", "f16")

# PE p-state warm-up reps: dependency-free 1-wide matmuls bridging the
# input-DMA window so real matmuls start at full clock.
WARMUP = int(os.environ.get("ATTN_WARMUP", "1300"))


def build_nc(mode=MODE):
    DT = {"f32r": F32R, "f32": F32, "f16": F16}[mode]
    ODT = F16 if mode == "f16" else F32
    nc = bacc.Bacc("TRN2", target_bir_lowering=False, debug=False)

    xT_d = nc.declare_dram_parameter("xT", [DIM, N], DT, isOutput=False)
    wqk_d = nc.declare_dram_parameter("w_qk", [DIM, 768], DT, isOutput=False)
    wv_d = nc.declare_dram_parameter("w_v", [DIM, 384], DT, isOutput=False)
    wo_d = nc.declare_dram_parameter("w_o", [384, DIM], DT, isOutput=False)
    out_d = nc.declare_dram_parameter("out", [N, DIM], ODT, isOutput=True)

    with tile.TileContext(nc) as tc:
        with ExitStack() as ctx:
            persist = ctx.enter_context(tc.tile_pool(name="persist", bufs=1))
            pt_pool = ctx.enter_context(tc.tile_pool(name="pt", bufs=6))
            stats = ctx.enter_context(tc.tile_pool(name="stats", bufs=3))
            outsb = ctx.enter_context(tc.tile_pool(name="outsb", bufs=4))
            # One PSUM pool, tag-sized: "s2" 2x[128,1024] (4 banks, score
            # double-tiles), "acc" 2x[65,512] (2, AV accumulators), "mm"
            # 2x[128,512] (2, everything else). 8 banks total.
            psum = ctx.enter_context(tc.tile_pool(name="psum", bufs=2, space="PSUM"))

            xT = persist.tile([128, 6, N], DT)
            wqk = persist.tile([128, 6, 768], DT)
            wv = persist.tile([128, 6, 384], DT)
            wo = persist.tile([128, 3, 768], DT)
            qkT = persist.tile([128, 6, N], DT)
            v_sb = persist.tile([128, 8, 6 * 65], DT)
            attT = persist.tile([128, 3, N], DT)
            # last unit's odd-head normalized rows, kept at partitions 0:64
            # so the final j=2 matmuls read them without a partition-shift DMA
            attT2_odd = persist.tile([64, 512], DT)
            wo2_odd = persist.tile([64, 768], DT)
            ones_sb = persist.tile([65, DH], DT)
            out_partial = persist.tile([128, 8, DIM], ODT)

            # Input DMAs. Transfers serialize on the shared ~360GB/s DMA
            # path, and each dma_start also costs ~625ns on the single
            # HWDGE descriptor generator — so batch the weight loads into
            # one DMA per tensor (per pair-block for w_qk; pair 0 first,
            # it gates the first scores). xT stays per-kt: each tile
            # releases the next accumulation step of the chasing
            # projections. Transfer-queue order = need order:
            # wqk_p0 | xT0..5 | wv | wqk_p1 | wqk_p2 | wo.
            def kpc(dram_ap):
                return dram_ap.rearrange("(k p) c -> p k c", p=128)

            nc.scalar.dma_start(out=wqk[:, :, 0:256], in_=kpc(wqk_d[:, 0:256]))
            for kt in range(6):
                nc.sync.dma_start(out=xT[:, kt, :], in_=xT_d[kt * 128:(kt + 1) * 128, :])
            nc.sync.dma_start(out=wv, in_=kpc(wv_d[:, :]))
            nc.sync.dma_start(out=wqk[:, :, 256:512], in_=kpc(wqk_d[:, 256:512]))
            nc.sync.dma_start(out=wqk[:, :, 512:768], in_=kpc(wqk_d[:, 512:768]))
            nc.sync.dma_start(out=wo, in_=kpc(wo_d[:, :]))
            # ones: v_sb[:, i, h*65 + 64] = 1.0 for all (i, h) (denominator
            # accumulator columns), and a partition-64 row of ones for the
            # denominator broadcast matmul. Constants — memset, no DMA.
            v_ones_view = v_sb.rearrange("p i (h c) -> p i h c", h=6)[:, :, :, 64]
            nc.gpsimd.memset(v_ones_view, 1.0)
            nc.gpsimd.memset(ones_sb[64:65, :], 1.0)
            # pair 2's odd-head w_o rows rebased to partitions 0:64 for the
            # tail's even/odd-split matmuls (Pool; off the startup path)
            nc.gpsimd.tensor_copy(wo2_odd, wo[64:128, 2, :])

            # PE clock warm-up: the tensor engine would otherwise idle
            # through the input-DMA window and start the projections at a
            # low p-state (the cost ramp needs ~3us of continuous busy).
            # Dependency-free 1-wide matmuls (~5ns each) bridge the window.
            warm_c = nc.const_aps.scalar_like(1.0, xT[:, 0, 0:1], dtype=F32)
            warm_ps = psum.tile([1, 1], F32, tag="mm", name="warm_ps")
            for _w in range(WARMUP):
                nc.tensor.matmul(warm_ps, warm_c, warm_c[0:128, 0:1],
                                 start=True, stop=True)

            def qk_pair0():
                """qkT tiles 0,1 (q,k of pair 0) — kt-major across all four
                (mt, chunk) accumulators so each arriving xT tile advances
                everything; after the last tile lands only one kt-step of
                work remains. Uses the two 2-bank score slots (idle until
                the weave starts)."""
                ps = {}
                for mt in (0, 1):
                    ps[mt] = psum.tile([128, 1024], F32, tag="s2",
                                       name=f"qk0_ps_{mt}")
                for kt in range(6):
                    for ch in (0, 1):  # ch0 first: it gates the first scores
                        for mt in (0, 1):
                            nc.tensor.matmul(
                                ps[mt][:, ch * 512:(ch + 1) * 512],
                                wqk[:, kt, mt * 128:(mt + 1) * 128],
                                xT[:, kt, ch * 512:(ch + 1) * 512],
                                start=(kt == 0),
                                stop=(kt == 5),
                            )
                # evict chunk-0 halves first (DVE+ACT in parallel) — the
                # first unit's scores only need them, not chunk 1
                nc.vector.tensor_copy(qkT[:, 0, 0:512], ps[0][:, 0:512])
                nc.scalar.copy(qkT[:, 1, 0:512], ps[1][:, 0:512])
                nc.vector.tensor_copy(qkT[:, 0, 512:1024], ps[0][:, 512:1024])
                nc.scalar.copy(qkT[:, 1, 512:1024], ps[1][:, 512:1024])

            qk_pair0()

            def qk_group(mt, chs=(0, 1)):
                """qkT[mt] = (w_qk col-block mt)^T @ xT, one 512-col chunk
                per call item. Col blocks (pair-packed): mt=2p -> q of pair
                p, 2p+1 -> k."""
                for ch in chs:
                    ps = psum.tile([128, 512], F32, tag="mm", name=f"qk_ps_{mt}_{ch}")
                    for kt in range(6):
                        nc.tensor.matmul(
                            ps,
                            wqk[:, kt, mt * 128:(mt + 1) * 128],
                            xT[:, kt, ch * 512:(ch + 1) * 512],
                            start=(kt == 0),
                            stop=(kt == 5),
                        )
                    nc.vector.tensor_copy(qkT[:, mt, ch * 512:(ch + 1) * 512], ps)

            def v_group(i):
                """v rows-block i = x[i-block] @ w_v, strided into v_sb"""
                ps = psum.tile([128, 384], F32, tag="mm", name=f"v_ps_{i}")
                for kt in range(6):
                    nc.tensor.matmul(
                        ps,
                        xT[:, kt, i * 128:(i + 1) * 128],
                        wv[:, kt, :],
                        start=(kt == 0),
                        stop=(kt == 5),
                    )
                dst = v_sb[:, i, :].rearrange("p (h c) -> p h c", h=6)[:, :, 0:DH]
                src = ps.rearrange("p (h c) -> p h c", h=6)
                nc.vector.tensor_copy(dst, src)

            def attention_unit(p, ch, emit_v, early=(), late=(), last=False):
                """Heads (2p, 2p+1), query chunk ch: both heads' scores into
                one 2-bank PSUM tile -> one 1024-wide exp per key block ->
                AV with fused denominator row -> normalize closures.
                Filler closures fill the tensor engine's exp-latency
                bubbles (the weave is ACT-bound): `early` (the previous
                unit's normalizes — they free the accumulator banks this
                unit's successor waits on) pops at steps 0-1; `late` (out
                rows that read the previous unit's partition-shift DMA,
                ~2.4us after its normalize) drains over steps 3-7."""
                early, late = list(early), list(late)
                qt = 2 * p       # qkT tile of this pair's q
                kt_ = 2 * p + 1  # qkT tile of this pair's k
                o_ps = {}
                for hp in range(2):
                    o_ps[hp] = psum.tile(
                        [65, 512], F32, tag="acc", name=f"oacc_{p}_{ch}_{hp}"
                    )
                for i in range(8):
                    s2 = psum.tile([128, 1024], F32, tag="s2",
                                   name=f"s_{p}_{ch}_{i}")
                    for hp in range(2):
                        lo, hi = hp * 64, hp * 64 + 64
                        nc.tensor.matmul(
                            s2[:, hp * 512:(hp + 1) * 512],
                            qkT[lo:hi, kt_, i * 128:(i + 1) * 128],
                            qkT[lo:hi, qt, ch * 512:(ch + 1) * 512],
                            start=True,
                            stop=True,
                        )
                    pt2 = pt_pool.tile([128, 1024], DT, tag="pt",
                                       name=f"pt_{p}_{ch}_{i}")
                    nc.scalar.activation(
                        pt2, s2, mybir.ActivationFunctionType.Exp, scale=SCALE,
                    )
                    # popped after the scores/exp emission so filler work
                    # never delays the ACT feed in the instruction stream.
                    # `late` spreads over steps 3-7 (mid-unit steps starve
                    # ~190ns/step without filler) but always keeps one item
                    # for step 7, where it lands between the unit's last
                    # scores and its exp-gated final AVs.
                    if early:
                        early.pop(0)()
                    elif i >= 3:
                        while late and len(late) > 7 - i:
                            late.pop(0)()
                        if late and i in (3, 5) and len(late) > (7 - i) // 2:
                            late.pop(0)()
                    if emit_v:
                        # emitted between scores and AV: fills the exp
                        # latency and keeps the w_v DMA off the scores path
                        v_group(i)
                    for hp in range(2):
                        h = 2 * p + hp
                        nc.tensor.matmul(
                            o_ps[hp],
                            v_sb[:, i, h * 65:h * 65 + 65],
                            pt2[:, hp * 512:(hp + 1) * 512],
                            start=(i == 0),
                            stop=(i == 7),
                        )
                # denominator reciprocals start NOW (DVE is off the critical
                # path here) so the accumulator banks free as soon as the
                # deferred bc+mult run in the next unit's filler slots.
                dinvs = {}
                for hp in range(2):
                    dinv = stats.tile(
                        [65, 512], DT, tag=f"dinv{hp}", name=f"dinv_{p}_{ch}_{hp}"
                    )
                    with nc.allow_low_precision(
                        reason="softmax denominators are O(100); rounding "
                        "of 1/denom is in line with the fp16 pipeline"
                    ):
                        nc.vector.reciprocal(dinv[64:65, :], o_ps[hp][64:65, :])
                    dinvs[hp] = dinv

                # normalize: attT rows [hp*64 : hp*64+64] of k-tile p, cols
                # ch. 1/denom is broadcast across partitions with a K=1
                # matmul (ones[1,64]^T @ dinv[1,512] -> [64,512] in PSUM),
                # evicted on Pool (an engine op can read only ONE PSUM
                # operand, and the multiply needs the accumulator). Returned
                # as closures deferred into the next unit's filler stream.
                def make_norm(hp):
                    def go():
                        # Evict the raw accumulator to fp16 SBUF on ACT
                        # first: the PSUM bank frees immediately (the next
                        # unit's AVs wait on it), and the normalize multiply
                        # becomes an all-16-bit SBUF op (DVE 2x mode).
                        acc_sb = stats.tile(
                            [64, 512], DT, tag=f"accsb{hp}", name=f"as_{p}_{ch}_{hp}"
                        )
                        nc.scalar.copy(acc_sb, o_ps[hp][0:64, :])
                        bc = psum.tile(
                            [64, 512], F32, tag="mm", name=f"bc_{p}_{ch}_{hp}"
                        )
                        nc.tensor.matmul(
                            bc, ones_sb[64:65, :], dinvs[hp][64:65, :],
                            start=True, stop=True,
                        )
                        bc_sb = stats.tile(
                            [64, 512], DT, tag="bc_sb", name=f"bcs_{p}_{ch}_{hp}"
                        )
                        # bc is PSUM (Pool can't read it): split DVE/ACT so
                        # neither chain serializes both heads
                        if hp == 0:
                            nc.vector.tensor_copy(bc_sb, bc)
                        else:
                            nc.scalar.copy(bc_sb, bc)
                        if hp == 0:
                            nc.vector.tensor_mul(
                                attT[0:64, p, ch * 512:(ch + 1) * 512],
                                acc_sb,
                                bc_sb,
                            )
                        elif last:
                            # keep at partitions 0:64; the tail's final
                            # matmuls read it directly (even/odd K-split)
                            nc.vector.tensor_mul(attT2_odd, acc_sb, bc_sb)
                        else:
                            tmp = stats.tile(
                                [64, 512], DT, tag="odd_tmp", name=f"ot_{p}_{ch}_{hp}"
                            )
                            nc.vector.tensor_mul(tmp, acc_sb, bc_sb)
                            # partition-shifting copy (rows 64:128) is DMA-only
                            nc.sync.dma_start(
                                out=attT[64:128, p, ch * 512:(ch + 1) * 512],
                                in_=tmp,
                            )
                    return go
                return [make_norm(0), make_norm(1)]

            def out_group(i, ch):
                """Half an out-projection row-block: matmuls + copy into the
                per-block staging tile; ch==1 flushes one 768-wide DMA."""
                c0, cw = ((0, 512), (512, 256))[ch]
                if ch == 0:
                    osb = outsb.tile([128, 768], ODT, tag="osb2", name=f"osb2_{i}")
                    _osb_cache[i] = osb
                else:
                    osb = _osb_cache.pop(i)
                ps = psum.tile([128, 512], F32, tag="mm", name=f"o_ps_{i}_{ch}")
                for j in range(3):
                    nc.tensor.matmul(
                        ps[:, 0:cw],
                        attT[:, j, i * 128:(i + 1) * 128],
                        wo[:, j, c0:c0 + cw],
                        start=(j == 0),
                        stop=(j == 2),
                    )
                nc.vector.tensor_copy(osb[:, c0:c0 + cw], ps[:, 0:cw])
                if ch == 1:
                    nc.sync.dma_start(
                        out=out_d[i * 128:(i + 1) * 128, :], in_=osb
                    )

            _osb_cache = {}

            def out_group_partial(i, ch):
                """j=0,1 of (row-block i, chunk ch) into the partial store
                (the finals add the j=2 term on top). Pool eviction: DVE is
                on the normalize chain in the tail."""
                c0, cw = ((0, 512), (512, 256))[ch]
                ps = psum.tile([128, 512], F32, tag="mm", name=f"pp_ps_{i}_{ch}")
                for j in range(2):
                    nc.tensor.matmul(
                        ps[:, 0:cw],
                        attT[:, j, i * 128:(i + 1) * 128],
                        wo[:, j, c0:c0 + cw],
                        start=(j == 0),
                        stop=(j == 1),
                    )
                nc.vector.tensor_copy(out_partial[:, i, c0:c0 + cw], ps[:, 0:cw])

            def out_rows_partial2(i):
                """j=0,1 of row-block i (both column halves) in one freed
                2-bank score slot, evicted with a single 768-wide Pool copy
                (DVE is on the normalize chain in the tail)."""
                ps = psum.tile([128, 1024], F32, tag="s2", name=f"pp_ps_{i}")
                for j in range(2):
                    for c0, cw in ((0, 512), (512, 256)):
                        nc.tensor.matmul(
                            ps[:, c0:c0 + cw],
                            attT[:, j, i * 128:(i + 1) * 128],
                            wo[:, j, c0:c0 + cw],
                            start=(j == 0),
                            stop=(j == 1),
                        )
                nc.scalar.copy(out_partial[:, i, :], ps[:, 0:768])  # ACT: idle tail

            def out_group_final2(i):
                """Row-block i of the tail: j=2 contribution split into the
                even head (attT partitions 0:64) and the odd head
                (attT2_odd, partitions 0:64 — no partition-shift DMA wait),
                added onto the precomputed j=0,1 partials, flushed per
                512/256 half so the last DMA is small and early. Uses the
                accumulator banks (freed by the last unit's normalizes);
                adds alternate DVE/Pool."""
                qs = (i - 4) * 128
                ptag = "acc" if i % 2 == 0 else "mm"
                osb = outsb.tile([128, 768], ODT, tag="osb2", name=f"osb2_{i}")
                for c0, cw in ((0, 512), (512, 256)):
                    ps = psum.tile([128, 512], F32, tag=ptag, name=f"f2_ps_{i}_{c0}")
                    nc.tensor.matmul(
                        ps[:, 0:cw],
                        attT[0:64, 2, i * 128:(i + 1) * 128],
                        wo[0:64, 2, c0:c0 + cw],
                        start=True, stop=False,
                    )
                    nc.tensor.matmul(
                        ps[:, 0:cw],
                        attT2_odd[:, qs:qs + 128],
                        wo2_odd[:, c0:c0 + cw],
                        start=False, stop=True,
                    )
                    nc.vector.tensor_add(
                        osb[:, c0:c0 + cw], ps[:, 0:cw],
                        out_partial[:, i, c0:c0 + cw],
                    )
                    nc.sync.dma_start(
                        out=out_d[i * 128:(i + 1) * 128, c0:c0 + cw],
                        in_=osb[:, c0:c0 + cw],
                    )

            # The weave. Query-chunk-0 units first: once (0,0),(1,0),(2,0)
            # are done, output row-blocks 0..3 are fully determined, so the
            # out-projection for rows 0-3 (and its DMA) spreads across the
            # chunk-1 units. Filler loads are balanced against each unit's
            # ~8.3us of exp work (u0 carries the v projection inherently).
            # qkT tiles feed the NEXT unit: pair 1 (tiles 2,3) from u0's
            # fillers, pair 2 (tiles 4,5) from u1's; chunk-1 q columns
            # (consumed only by the chunk-1 units) trail in u1/u2.
            nrm = attention_unit(0, 0, emit_v=True, late=[
                lambda: qk_group(3, chs=(0,)),
                lambda: qk_group(2, chs=(0,)),
                lambda: qk_group(3, chs=(1,)),
            ])
            nrm = attention_unit(1, 0, emit_v=False, early=nrm, late=[
                lambda: qk_group(5, chs=(0,)),
                lambda: qk_group(4, chs=(0,)),
                lambda: qk_group(5, chs=(1,)),
            ])
            nrm = attention_unit(2, 0, emit_v=False, early=nrm, late=[
                lambda: qk_group(2, chs=(1,)),
                lambda: qk_group(4, chs=(1,)),
            ])
            nrm = attention_unit(0, 1, emit_v=False, early=nrm, late=[
                lambda i=i, ch=c: out_group(i, ch)
                for (i, c) in ((0, 0), (0, 1), (1, 0))
            ])
            nrm = attention_unit(1, 1, emit_v=False, early=nrm, late=[
                lambda i=i, ch=c: out_group(i, ch)
                for (i, c) in ((1, 1), (2, 0), (2, 1))
            ])
            nrm = attention_unit(2, 1, emit_v=False, early=nrm, late=[
                lambda i=i, ch=c: out_group(i, ch)
                for (i, c) in ((3, 0), (3, 1))
            ] + [
                lambda i=i, ch=c: out_group_partial(i, ch)
                for i in (4, 5) for c in (0, 1)
            ], last=True)
            # Pipelined tail: rows 4,5's j=0,1 partials ran as the last
            # unit's late fillers (pairs 0,1 chunk 1 normalized by then);
            # rows 6,7's fill the normalize-chain latency here; finals then
            # stream one half-row DMA at a time.
            nrm[0]()
            out_rows_partial2(6)
            out_rows_partial2(7)
            nrm[1]()
            for i in range(4, 8):
                out_group_final2(i)

    nc.compile()
    return nc


_NC_CACHE = {}


def _get_nc():
    if MODE not in _NC_CACHE:
        _NC_CACHE[MODE] = build_nc(MODE)
    return _NC_CACHE[MODE]


def kernel(x, w_qkv, w_out, b_out):
    x = np.asarray(x, dtype=np.float32)
    w_qkv = np.asarray(w_qkv, dtype=np.float32)
    w_out = np.asarray(w_out, dtype=np.float32)
    b_out = np.asarray(b_out, dtype=np.float32)

    nc = _get_nc()
    if MODE == "f16":
        x = x.astype(np.float16)
        w_qkv = w_qkv.astype(np.float16)
        w_out = w_out.astype(np.float16)
    in_maps = []
    for c in range(NCORES):
        b = c // 2
        hs = (c % 2) * HEADS_PER_CORE
        q_cols = w_qkv[:, hs * DH:(hs + 6) * DH]
        k_cols = w_qkv[:, 768 + hs * DH:768 + (hs + 6) * DH]
        # pair-packed: [q_p0 | k_p0 | q_p1 | k_p1 | q_p2 | k_p2], 128 each
        wqk_packed = np.concatenate(
            [blk for p in range(3)
             for blk in (q_cols[:, p * 128:(p + 1) * 128],
                         k_cols[:, p * 128:(p + 1) * 128])],
            axis=1,
        )
        in_maps.append({
            "xT": np.ascontiguousarray(x[b].T),
            "w_qk": np.ascontiguousarray(wqk_packed),
            "w_v": np.ascontiguousarray(w_qkv[:, 1536 + hs * DH:1536 + (hs + 6) * DH]),
            "w_o": np.ascontiguousarray(w_out[hs * DH:(hs + 6) * DH, :]),
        })

    res = run_bass_kernel_spmd(nc, in_maps, core_ids=list(range(NCORES))).results

    out = np.empty((4, N, DIM), dtype=np.float32)
    for b in range(4):
        out[b] = (res[2 * b]["out"].astype(np.float32)
                  + res[2 * b + 1]["out"].astype(np.float32) + b_out)
    return out


# revision 39
# speedup vs baseline: 1.0228x; 1.0228x over previous
"""Multi-head attention forward (B=4, N=1024, D=768, H=12, dh=64) on 8 TRN2 cores.

Sharding: (batch, head-group) — core c handles batch b = c//2 and heads
hs..hs+5 where hs = (c%2)*6.  Each core computes its 6 heads' contribution
to out[b] = attn(x[b]) @ W_out_rows(for its heads); host sums the two
partials per batch and adds the bias (the "all-reduce after final linear").

Per-core dataflow (all contraction dims on SBUF partitions), fp16 wire
dtype (host casts; fp32 PSUM accumulation; ~1e-3 end-to-end rel err):
  qkT  [768,1024] = w_qk^T @ x^T          (d-major q,k — feeds scores;
                                           w_qk cols pair-packed
                                           [q_p0|k_p0|q_p1|k_p1|q_p2|k_p2])
  v    [1024,390] = x @ w_v (+ ones col)  (n-major v — feeds AV^T)
  S^T  [1024,1024]/head = k_h @ q_h^T     (keys on partitions, 2 heads
                                           row-packed in the PE array,
                                           both heads' scores in one
                                           2-bank PSUM tile so each
                                           (i, pair) step is ONE 1024-wide
                                           exp — ACT is the weave pacer)
  P^T  = exp(S^T * scale)                 (no max-sub: scores ~ N(0,1))
  oT   [65,1024]/head = [v_h|1]^T @ P^T   (row 64 = softmax denominators)
  attT = oT[0:64] * (1/denom)             (K=1 matmul broadcasts 1/denom
                                           across partitions, DVE mult)
  out  [1024,768] = attT^T @ w_o          (partial; host all-reduce)

The weave: filler work (projections for later pairs, v blocks, finished
output rows) is interleaved into each attention unit's emission stream so
the tensor engine's exp-latency bubbles are filled, balanced against the
unit's ~8.3us of ACT exp work. The last unit ends with a pipelined tail:
row partials first, then the normalize chain split even/odd so the final
j=2 matmuls read the odd-head tile directly (no SBUF->SBUF partition-shift
DMA on the critical path), one output DMA per finished half-row.
"""
import os
import sys

sys.path.insert(0, "/opt/trn_rl_repo")

# The kernel needs the axon-tunneled TRN2 PJRT backend; a JAX_PLATFORMS=cpu
# pin (common for reference-side jax) would hide the NeuronCores.
if os.environ.get("JAX_PLATFORMS", "").strip() == "cpu":
    del os.environ["JAX_PLATFORMS"]

import numpy as np
import concourse.bass as bass
import concourse.bacc as bacc
import concourse.tile as tile
from concourse import mybir
from concourse.bass_utils import run_bass_kernel_spmd
from contextlib import ExitStack

F32 = mybir.dt.float32
F32R = mybir.dt.float32r
F16 = mybir.dt.float16

DIM = 768
N = 1024
HEADS_PER_CORE = 6
DH = 64
SCALE = DH ** -0.5
NCORES = 8

# "f16"  = fp16 pipeline (1 cyc/row PE at any width, half DMA traffic;
#          ~1e-3 end-to-end rel err)
# "f32r" = TF32-like matmul mode (~4e-4 end-to-end rel err)
MODE = os.environ.get("ATTN_MM_DTYPE", "f16")

# PE p-state warm-up reps: dependency-free 1-wide matmuls bridging the
# input-DMA window so real matmuls start at full clock.
WARMUP = int(os.environ.get("ATTN_WARMUP", "1300"))


def build_nc(mode=MODE):
    DT = {"f32r": F32R, "f32": F32, "f16": F16}[mode]
    ODT = F16 if mode == "f16" else F32
    nc = bacc.Bacc("TRN2", target_bir_lowering=False, debug=False)

    xT_d = nc.declare_dram_parameter("xT", [DIM, N], DT, isOutput=False)
    wqk_d = nc.declare_dram_parameter("w_qk", [DIM, 768], DT, isOutput=False)
    wv_d = nc.declare_dram_parameter("w_v", [DIM, 384], DT, isOutput=False)
    wo_d = nc.declare_dram_parameter("w_o", [384, DIM], DT, isOutput=False)
    out_d = nc.declare_dram_parameter("out", [N, DIM], ODT, isOutput=True)

    with tile.TileContext(nc) as tc:
        with ExitStack() as ctx:
            persist = ctx.enter_context(tc.tile_pool(name="persist", bufs=1))
            pt_pool = ctx.enter_context(tc.tile_pool(name="pt", bufs=6))
            stats = ctx.enter_context(tc.tile_pool(name="stats", bufs=3))
            outsb = ctx.enter_context(tc.tile_pool(name="outsb", bufs=4))
            # One PSUM pool, tag-sized: "s2" 2x[128,1024] (4 banks, score
            # double-tiles), "acc" 2x[65,512] (2, AV accumulators), "mm"
            # 2x[128,512] (2, everything else). 8 banks total.
            psum = ctx.enter_context(tc.tile_pool(name="psum", bufs=2, space="PSUM"))

            xT = persist.tile([128, 6, N], DT)
            wqk = persist.tile([128, 6, 768], DT)
            wv = persist.tile([128, 6, 384], DT)
            wo = persist.tile([128, 3, 768], DT)
            qkT = persist.tile([128, 6, N], DT)
            v_sb = persist.tile([128, 8, 6 * 65], DT)
            attT = persist.tile([128, 3, N], DT)
            # last unit's odd-head normalized rows, kept at partitions 0:64
            # so the final j=2 matmuls read them without a partition-shift DMA
            attT2_odd = persist.tile([64, 512], DT)
            wo2_odd = persist.tile([64, 768], DT)
            ones_sb = persist.tile([65, DH], DT)
            out_partial = persist.tile([128, 8, DIM], ODT)

            # Input DMAs. Transfers serialize on the shared ~360GB/s DMA
            # path, and each dma_start also costs ~625ns on the single
            # HWDGE descriptor generator — so batch the weight loads into
            # one DMA per tensor (per pair-block for w_qk; pair 0 first,
            # it gates the first scores). xT stays per-kt: each tile
            # releases the next accumulation step of the chasing
            # projections. Transfer-queue order = need order:
            # wqk_p0 | xT0..5 | wv | wqk_p1 | wqk_p2 | wo.
            def kpc(dram_ap):
                return dram_ap.rearrange("(k p) c -> p k c", p=128)

            nc.scalar.dma_start(out=wqk[:, :, 0:256], in_=kpc(wqk_d[:, 0:256]))
            for kt in range(6):
                nc.sync.dma_start(out=xT[:, kt, :], in_=xT_d[kt * 128:(kt + 1) * 128, :])
            nc.sync.dma_start(out=wv, in_=kpc(wv_d[:, :]))
            nc.sync.dma_start(out=wqk[:, :, 256:512], in_=kpc(wqk_d[:, 256:512]))
            nc.sync.dma_start(out=wqk[:, :, 512:768], in_=kpc(wqk_d[:, 512:768]))
            nc.sync.dma_start(out=wo, in_=kpc(wo_d[:, :]))
            # ones: v_sb[:, i, h*65 + 64] = 1.0 for all (i, h) (denominator
            # accumulator columns), and a partition-64 row of ones for the
            # denominator broadcast matmul. Constants — memset, no DMA.
            v_ones_view = v_sb.rearrange("p i (h c) -> p i h c", h=6)[:, :, :, 64]
            nc.gpsimd.memset(v_ones_view, 1.0)
            nc.gpsimd.memset(ones_sb[64:65, :], 1.0)
            # pair 2's odd-head w_o rows rebased to partitions 0:64 for the
            # tail's even/odd-split matmuls (Pool; off the startup path)
            nc.gpsimd.tensor_copy(wo2_odd, wo[64:128, 2, :])

            # PE clock warm-up: the tensor engine would otherwise idle
            # through the input-DMA window and start the projections at a
            # low p-state (the cost ramp needs ~3us of continuous busy).
            # Dependency-free 1-wide matmuls (~5ns each) bridge the window.
            warm_c = nc.const_aps.scalar_like(1.0, xT[:, 0, 0:1], dtype=F32)
            warm_ps = psum.tile([1, 1], F32, tag="mm", name="warm_ps")
            for _w in range(WARMUP):
                nc.tensor.matmul(warm_ps, warm_c, warm_c[0:128, 0:1],
                                 start=True, stop=True)

            def qk_pair0():
                """qkT tiles 0,1 (q,k of pair 0) — kt-major across all four
                (mt, chunk) accumulators so each arriving xT tile advances
                everything; after the last tile lands only one kt-step of
                work remains. Uses the two 2-bank score slots (idle until
                the weave starts)."""
                ps = {}
                for mt in (0, 1):
                    ps[mt] = psum.tile([128, 1024], F32, tag="s2",
                                       name=f"qk0_ps_{mt}")
                for kt in range(6):
                    for ch in (0, 1):  # ch0 first: it gates the first scores
                        for mt in (0, 1):
                            nc.tensor.matmul(
                                ps[mt][:, ch * 512:(ch + 1) * 512],
                                wqk[:, kt, mt * 128:(mt + 1) * 128],
                                xT[:, kt, ch * 512:(ch + 1) * 512],
                                start=(kt == 0),
                                stop=(kt == 5),
                            )
                # evict chunk-0 halves first (DVE+ACT in parallel) — the
                # first unit's scores only need them, not chunk 1
                nc.vector.tensor_copy(qkT[:, 0, 0:512], ps[0][:, 0:512])
                nc.scalar.copy(qkT[:, 1, 0:512], ps[1][:, 0:512])
                nc.vector.tensor_copy(qkT[:, 0, 512:1024], ps[0][:, 512:1024])
                nc.scalar.copy(qkT[:, 1, 512:1024], ps[1][:, 512:1024])

            qk_pair0()

            def qk_group(mt, chs=(0, 1)):
                """qkT[mt] = (w_qk col-block mt)^T @ xT, one 512-col chunk
                per call item. Col blocks (pair-packed): mt=2p -> q of pair
                p, 2p+1 -> k."""
                for ch in chs:
                    ps = psum.tile([128, 512], F32, tag="mm", name=f"qk_ps_{mt}_{ch}")
                    for kt in range(6):
                        nc.tensor.matmul(
                            ps,
                            wqk[:, kt, mt * 128:(mt + 1) * 128],
                            xT[:, kt, ch * 512:(ch + 1) * 512],
                            start=(kt == 0),
                            stop=(kt == 5),
                        )
                    nc.vector.tensor_copy(qkT[:, mt, ch * 512:(ch + 1) * 512], ps)

            def v_group(i):
                """v rows-block i = x[i-block] @ w_v, strided into v_sb"""
                ps = psum.tile([128, 384], F32, tag="mm", name=f"v_ps_{i}")
                for kt in range(6):
                    nc.tensor.matmul(
                        ps,
                        xT[:, kt, i * 128:(i + 1) * 128],
                        wv[:, kt, :],
                        start=(kt == 0),
                        stop=(kt == 5),
                    )
                dst = v_sb[:, i, :].rearrange("p (h c) -> p h c", h=6)[:, :, 0:DH]
                src = ps.rearrange("p (h c) -> p h c", h=6)
                nc.vector.tensor_copy(dst, src)

            def attention_unit(p, ch, emit_v, early=(), late=(), last=False):
                """Heads (2p, 2p+1), query chunk ch: both heads' scores into
                one 2-bank PSUM tile -> one 1024-wide exp per key block ->
                AV with fused denominator row -> normalize closures.
                Filler closures fill the tensor engine's exp-latency
                bubbles (the weave is ACT-bound): `early` (the previous
                unit's normalizes — they free the accumulator banks this
                unit's successor waits on) pops at steps 0-1; `late` (out
                rows that read the previous unit's partition-shift DMA,
                ~2.4us after its normalize) drains over steps 3-7."""
                early, late = list(early), list(late)
                qt = 2 * p       # qkT tile of this pair's q
                kt_ = 2 * p + 1  # qkT tile of this pair's k
                o_ps = {}
                for hp in range(2):
                    o_ps[hp] = psum.tile(
                        [65, 512], F32, tag="acc", name=f"oacc_{p}_{ch}_{hp}"
                    )
                for i in range(8):
                    s2 = psum.tile([128, 1024], F32, tag="s2",
                                   name=f"s_{p}_{ch}_{i}")
                    for hp in range(2):
                        lo, hi = hp * 64, hp * 64 + 64
                        nc.tensor.matmul(
                            s2[:, hp * 512:(hp + 1) * 512],
                            qkT[lo:hi, kt_, i * 128:(i + 1) * 128],
                            qkT[lo:hi, qt, ch * 512:(ch + 1) * 512],
                            start=True,
                            stop=True,
                        )
                    pt2 = pt_pool.tile([128, 1024], DT, tag="pt",
                                       name=f"pt_{p}_{ch}_{i}")
                    nc.scalar.activation(
                        pt2, s2, mybir.ActivationFunctionType.Exp, scale=SCALE,
                    )
                    # popped after the scores/exp emission so filler work
                    # never delays the ACT feed in the instruction stream.
                    # `late` spreads over steps 3-7 (mid-unit steps starve
                    # ~190ns/step without filler) but always keeps one item
                    # for step 7, where it lands between the unit's last
                    # scores and its exp-gated final AVs.
                    if early:
                        early.pop(0)()
                    elif i >= 3:
                        while late and len(late) > 7 - i:
                            late.pop(0)()
                        if late and i in (3, 5) and len(late) > (7 - i) // 2:
                            late.pop(0)()
                    if emit_v:
                        # emitted between scores and AV: fills the exp
                        # latency and keeps the w_v DMA off the scores path
                        v_group(i)
                    for hp in range(2):
                        h = 2 * p + hp
                        nc.tensor.matmul(
                            o_ps[hp],
                            v_sb[:, i, h * 65:h * 65 + 65],
                            pt2[:, hp * 512:(hp + 1) * 512],
                            start=(i == 0),
                            stop=(i == 7),
                        )
                # denominator reciprocals start NOW (DVE is off the critical
                # path here) so the accumulator banks free as soon as the
                # deferred bc+mult run in the next unit's filler slots.
                dinvs = {}
                for hp in range(2):
                    dinv = stats.tile(
                        [65, 512], DT, tag=f"dinv{hp}", name=f"dinv_{p}_{ch}_{hp}"
                    )
                    with nc.allow_low_precision(
                        reason="softmax denominators are O(100); rounding "
                        "of 1/denom is in line with the fp16 pipeline"
                    ):
                        nc.vector.reciprocal(dinv[64:65, :], o_ps[hp][64:65, :])
                    dinvs[hp] = dinv

                # normalize: attT rows [hp*64 : hp*64+64] of k-tile p, cols
                # ch. 1/denom is broadcast across partitions with a K=1
                # matmul (ones[1,64]^T @ dinv[1,512] -> [64,512] in PSUM),
                # evicted on Pool (an engine op can read only ONE PSUM
                # operand, and the multiply needs the accumulator). Returned
                # as closures deferred into the next unit's filler stream.
                def make_norm(hp):
                    def go():
                        acc = o_ps[hp]
                        if last:
                            # Tail only (ACT is idle then): evict the raw
                            # accumulator to fp16 SBUF on ACT so the PSUM
                            # bank frees immediately (the finals reuse it)
                            # and the multiply becomes an all-16-bit SBUF
                            # op (DVE 2x mode). Mid-weave this stretches
                            # the ACT-paced exp stream — not worth it.
                            acc_in = stats.tile(
                                [64, 512], DT, tag=f"accsb{hp}",
                                name=f"as_{p}_{ch}_{hp}"
                            )
                            nc.scalar.copy(acc_in, acc[0:64, :])
                        else:
                            acc_in = acc[0:64, :]
                        bc = psum.tile(
                            [64, 512], F32, tag="mm", name=f"bc_{p}_{ch}_{hp}"
                        )
                        nc.tensor.matmul(
                            bc, ones_sb[64:65, :], dinvs[hp][64:65, :],
                            start=True, stop=True,
                        )
                        bc_sb = stats.tile(
                            [64, 512], DT, tag="bc_sb", name=f"bcs_{p}_{ch}_{hp}"
                        )
                        # bc is PSUM (Pool can't read it): split DVE/ACT so
                        # neither chain serializes both heads
                        if hp == 0:
                            nc.vector.tensor_copy(bc_sb, bc)
                        else:
                            nc.scalar.copy(bc_sb, bc)
                        if hp == 0:
                            nc.vector.tensor_mul(
                                attT[0:64, p, ch * 512:(ch + 1) * 512],
                                acc_in,
                                bc_sb,
                            )
                        elif last:
                            # keep at partitions 0:64; the tail's final
                            # matmuls read it directly (even/odd K-split)
                            nc.vector.tensor_mul(attT2_odd, acc_in, bc_sb)
                        else:
                            tmp = stats.tile(
                                [64, 512], DT, tag="odd_tmp", name=f"ot_{p}_{ch}_{hp}"
                            )
                            nc.vector.tensor_mul(tmp, acc_in, bc_sb)
                            # partition-shifting copy (rows 64:128) is DMA-only
                            nc.sync.dma_start(
                                out=attT[64:128, p, ch * 512:(ch + 1) * 512],
                                in_=tmp,
                            )
                    return go
                return [make_norm(0), make_norm(1)]

            def out_group(i, ch):
                """Half an out-projection row-block: matmuls + copy into the
                per-block staging tile; ch==1 flushes one 768-wide DMA."""
                c0, cw = ((0, 512), (512, 256))[ch]
                if ch == 0:
                    osb = outsb.tile([128, 768], ODT, tag="osb2", name=f"osb2_{i}")
                    _osb_cache[i] = osb
                else:
                    osb = _osb_cache.pop(i)
                ps = psum.tile([128, 512], F32, tag="mm", name=f"o_ps_{i}_{ch}")
                for j in range(3):
                    nc.tensor.matmul(
                        ps[:, 0:cw],
                        attT[:, j, i * 128:(i + 1) * 128],
                        wo[:, j, c0:c0 + cw],
                        start=(j == 0),
                        stop=(j == 2),
                    )
                nc.vector.tensor_copy(osb[:, c0:c0 + cw], ps[:, 0:cw])
                if ch == 1:
                    nc.sync.dma_start(
                        out=out_d[i * 128:(i + 1) * 128, :], in_=osb
                    )

            _osb_cache = {}

            def out_group_partial(i, ch):
                """j=0,1 of (row-block i, chunk ch) into the partial store
                (the finals add the j=2 term on top). Pool eviction: DVE is
                on the normalize chain in the tail."""
                c0, cw = ((0, 512), (512, 256))[ch]
                ps = psum.tile([128, 512], F32, tag="mm", name=f"pp_ps_{i}_{ch}")
                for j in range(2):
                    nc.tensor.matmul(
                        ps[:, 0:cw],
                        attT[:, j, i * 128:(i + 1) * 128],
                        wo[:, j, c0:c0 + cw],
                        start=(j == 0),
                        stop=(j == 1),
                    )
                nc.vector.tensor_copy(out_partial[:, i, c0:c0 + cw], ps[:, 0:cw])

            def out_rows_partial2(i):
                """j=0,1 of row-block i (both column halves) in one freed
                2-bank score slot, evicted with a single 768-wide Pool copy
                (DVE is on the normalize chain in the tail)."""
                ps = psum.tile([128, 1024], F32, tag="s2", name=f"pp_ps_{i}")
                for j in range(2):
                    for c0, cw in ((0, 512), (512, 256)):
                        nc.tensor.matmul(
                            ps[:, c0:c0 + cw],
                            attT[:, j, i * 128:(i + 1) * 128],
                            wo[:, j, c0:c0 + cw],
                            start=(j == 0),
                            stop=(j == 1),
                        )
                nc.scalar.copy(out_partial[:, i, :], ps[:, 0:768])  # ACT: idle tail

            def out_group_final2(i):
                """Row-block i of the tail: j=2 contribution split into the
                even head (attT partitions 0:64) and the odd head
                (attT2_odd, partitions 0:64 — no partition-shift DMA wait),
                added onto the precomputed j=0,1 partials, flushed per
                512/256 half so the last DMA is small and early. Uses the
                accumulator banks (freed by the last unit's normalizes);
                adds alternate DVE/Pool."""
                qs = (i - 4) * 128
                ptag = "acc" if i % 2 == 0 else "mm"
                osb = outsb.tile([128, 768], ODT, tag="osb2", name=f"osb2_{i}")
                for c0, cw in ((0, 512), (512, 256)):
                    ps = psum.tile([128, 512], F32, tag=ptag, name=f"f2_ps_{i}_{c0}")
                    nc.tensor.matmul(
                        ps[:, 0:cw],
                        attT[0:64, 2, i * 128:(i + 1) * 128],
                        wo[0:64, 2, c0:c0 + cw],
                        start=True, stop=False,
                    )
                    nc.tensor.matmul(
                        ps[:, 0:cw],
                        attT2_odd[:, qs:qs + 128],
                        wo2_odd[:, c0:c0 + cw],
                        start=False, stop=True,
                    )
                    nc.vector.tensor_add(
                        osb[:, c0:c0 + cw], ps[:, 0:cw],
                        out_partial[:, i, c0:c0 + cw],
                    )
                    nc.sync.dma_start(
                        out=out_d[i * 128:(i + 1) * 128, c0:c0 + cw],
                        in_=osb[:, c0:c0 + cw],
                    )

            # The weave. Query-chunk-0 units first: once (0,0),(1,0),(2,0)
            # are done, output row-blocks 0..3 are fully determined, so the
            # out-projection for rows 0-3 (and its DMA) spreads across the
            # chunk-1 units. Filler loads are balanced against each unit's
            # ~8.3us of exp work (u0 carries the v projection inherently).
            # qkT tiles feed the NEXT unit: pair 1 (tiles 2,3) from u0's
            # fillers, pair 2 (tiles 4,5) from u1's; chunk-1 q columns
            # (consumed only by the chunk-1 units) trail in u1/u2.
            nrm = attention_unit(0, 0, emit_v=True, late=[
                lambda: qk_group(3, chs=(0,)),
                lambda: qk_group(2, chs=(0,)),
                lambda: qk_group(3, chs=(1,)),
            ])
            nrm = attention_unit(1, 0, emit_v=False, early=nrm, late=[
                lambda: qk_group(5, chs=(0,)),
                lambda: qk_group(4, chs=(0,)),
                lambda: qk_group(5, chs=(1,)),
            ])
            nrm = attention_unit(2, 0, emit_v=False, early=nrm, late=[
                lambda: qk_group(2, chs=(1,)),
                lambda: qk_group(4, chs=(1,)),
            ])
            nrm = attention_unit(0, 1, emit_v=False, early=nrm, late=[
                lambda i=i, ch=c: out_group(i, ch)
                for (i, c) in ((0, 0), (0, 1), (1, 0))
            ])
            nrm = attention_unit(1, 1, emit_v=False, early=nrm, late=[
                lambda i=i, ch=c: out_group(i, ch)
                for (i, c) in ((1, 1), (2, 0), (2, 1))
            ])
            nrm = attention_unit(2, 1, emit_v=False, early=nrm, late=[
                lambda i=i, ch=c: out_group(i, ch)
                for (i, c) in ((3, 0), (3, 1))
            ] + [
                lambda i=i, ch=c: out_group_partial(i, ch)
                for i in (4, 5) for c in (0, 1)
            ], last=True)
            # Pipelined tail: rows 4,5's j=0,1 partials ran as the last
            # unit's late fillers (pairs 0,1 chunk 1 normalized by then);
            # rows 6,7's fill the normalize-chain latency here; finals then
            # stream one half-row DMA at a time.
            nrm[0]()
            out_rows_partial2(6)
            out_rows_partial2(7)
            nrm[1]()
            for i in range(4, 8):
                out_group_final2(i)

    nc.compile()
    return nc


_NC_CACHE = {}


def _get_nc():
    if MODE not in _NC_CACHE:
        _NC_CACHE[MODE] = build_nc(MODE)
    return _NC_CACHE[MODE]


def kernel(x, w_qkv, w_out, b_out):
    x = np.asarray(x, dtype=np.float32)
    w_qkv = np.asarray(w_qkv, dtype=np.float32)
    w_out = np.asarray(w_out, dtype=np.float32)
    b_out = np.asarray(b_out, dtype=np.float32)

    nc = _get_nc()
    if MODE == "f16":
        x = x.astype(np.float16)
        w_qkv = w_qkv.astype(np.float16)
        w_out = w_out.astype(np.float16)
    in_maps = []
    for c in range(NCORES):
        b = c // 2
        hs = (c % 2) * HEADS_PER_CORE
        q_cols = w_qkv[:, hs * DH:(hs + 6) * DH]
        k_cols = w_qkv[:, 768 + hs * DH:768 + (hs + 6) * DH]
        # pair-packed: [q_p0 | k_p0 | q_p1 | k_p1 | q_p2 | k_p2], 128 each
        wqk_packed = np.concatenate(
            [blk for p in range(3)
             for blk in (q_cols[:, p * 128:(p + 1) * 128],
                         k_cols[:, p * 128:(p + 1) * 128])],
            axis=1,
        )
        in_maps.append({
            "xT": np.ascontiguousarray(x[b].T),
            "w_qk": np.ascontiguousarray(wqk_packed),
            "w_v": np.ascontiguousarray(w_qkv[:, 1536 + hs * DH:1536 + (hs + 6) * DH]),
            "w_o": np.ascontiguousarray(w_out[hs * DH:(hs + 6) * DH, :]),
        })

    res = run_bass_kernel_spmd(nc, in_maps, core_ids=list(range(NCORES))).results

    out = np.empty((4, N, DIM), dtype=np.float32)
    for b in range(4):
        out[b] = (res[2 * b]["out"].astype(np.float32)
                  + res[2 * b + 1]["out"].astype(np.float32) + b_out)
    return out


# revision 40
# speedup vs baseline: 1.0290x; 1.0061x over previous
"""Multi-head attention forward (B=4, N=1024, D=768, H=12, dh=64) on 8 TRN2 cores.

Sharding: (batch, head-group) — core c handles batch b = c//2 and heads
hs..hs+5 where hs = (c%2)*6.  Each core computes its 6 heads' contribution
to out[b] = attn(x[b]) @ W_out_rows(for its heads); host sums the two
partials per batch and adds the bias (the "all-reduce after final linear").

Per-core dataflow (all contraction dims on SBUF partitions), fp16 wire
dtype (host casts; fp32 PSUM accumulation; ~1e-3 end-to-end rel err):
  qkT  [768,1024] = w_qk^T @ x^T          (d-major q,k — feeds scores;
                                           w_qk cols pair-packed
                                           [q_p0|k_p0|q_p1|k_p1|q_p2|k_p2])
  v    [1024,390] = x @ w_v (+ ones col)  (n-major v — feeds AV^T)
  S^T  [1024,1024]/head = k_h @ q_h^T     (keys on partitions, 2 heads
                                           row-packed in the PE array,
                                           both heads' scores in one
                                           2-bank PSUM tile so each
                                           (i, pair) step is ONE 1024-wide
                                           exp — ACT is the weave pacer)
  P^T  = exp(S^T * scale)                 (no max-sub: scores ~ N(0,1))
  oT   [65,1024]/head = [v_h|1]^T @ P^T   (row 64 = softmax denominators)
  attT = oT[0:64] * (1/denom)             (K=1 matmul broadcasts 1/denom
                                           across partitions, DVE mult)
  out  [1024,768] = attT^T @ w_o          (partial; host all-reduce)

The weave: filler work (projections for later pairs, v blocks, finished
output rows) is interleaved into each attention unit's emission stream so
the tensor engine's exp-latency bubbles are filled, balanced against the
unit's ~8.3us of ACT exp work. The last unit ends with a pipelined tail:
row partials first, then the normalize chain split even/odd so the final
j=2 matmuls read the odd-head tile directly (no SBUF->SBUF partition-shift
DMA on the critical path), one output DMA per finished half-row.
"""
import os
import sys

sys.path.insert(0, "/opt/trn_rl_repo")

# The kernel needs the axon-tunneled TRN2 PJRT backend; a JAX_PLATFORMS=cpu
# pin (common for reference-side jax) would hide the NeuronCores.
if os.environ.get("JAX_PLATFORMS", "").strip() == "cpu":
    del os.environ["JAX_PLATFORMS"]

import numpy as np
import concourse.bass as bass
import concourse.bacc as bacc
import concourse.tile as tile
from concourse import mybir
from concourse.bass_utils import run_bass_kernel_spmd
from contextlib import ExitStack

F32 = mybir.dt.float32
F32R = mybir.dt.float32r
F16 = mybir.dt.float16

DIM = 768
N = 1024
HEADS_PER_CORE = 6
DH = 64
SCALE = DH ** -0.5
NCORES = 8

# "f16"  = fp16 pipeline (1 cyc/row PE at any width, half DMA traffic;
#          ~1e-3 end-to-end rel err)
# "f32r" = TF32-like matmul mode (~4e-4 end-to-end rel err)
MODE = os.environ.get("ATTN_MM_DTYPE", "f16")

# PE p-state warm-up reps: dependency-free 1-wide matmuls bridging the
# input-DMA window so real matmuls start at full clock.
WARMUP = int(os.environ.get("ATTN_WARMUP", "1300"))


def build_nc(mode=MODE):
    DT = {"f32r": F32R, "f32": F32, "f16": F16}[mode]
    ODT = F16 if mode == "f16" else F32
    nc = bacc.Bacc("TRN2", target_bir_lowering=False, debug=False)

    xT_d = nc.declare_dram_parameter("xT", [DIM, N], DT, isOutput=False)
    wqk_d = nc.declare_dram_parameter("w_qk", [DIM, 768], DT, isOutput=False)
    wv_d = nc.declare_dram_parameter("w_v", [DIM, 384], DT, isOutput=False)
    wo_d = nc.declare_dram_parameter("w_o", [384, DIM], DT, isOutput=False)
    out_d = nc.declare_dram_parameter("out", [N, DIM], ODT, isOutput=True)

    with tile.TileContext(nc) as tc:
        with ExitStack() as ctx:
            persist = ctx.enter_context(tc.tile_pool(name="persist", bufs=1))
            pt_pool = ctx.enter_context(tc.tile_pool(name="pt", bufs=6))
            stats = ctx.enter_context(tc.tile_pool(name="stats", bufs=3))
            outsb = ctx.enter_context(tc.tile_pool(name="outsb", bufs=4))
            # One PSUM pool, tag-sized: "s2" 2x[128,1024] (4 banks, score
            # double-tiles), "acc" 2x[65,512] (2, AV accumulators), "mm"
            # 2x[128,512] (2, everything else). 8 banks total.
            psum = ctx.enter_context(tc.tile_pool(name="psum", bufs=2, space="PSUM"))

            xT = persist.tile([128, 6, N], DT)
            wqk = persist.tile([128, 6, 768], DT)
            wv = persist.tile([128, 6, 384], DT)
            wo = persist.tile([128, 3, 768], DT)
            qkT = persist.tile([128, 6, N], DT)
            v_sb = persist.tile([128, 8, 6 * 65], DT)
            attT = persist.tile([128, 3, N], DT)
            # last unit's odd-head normalized rows, kept at partitions 0:64
            # so the final j=2 matmuls read them without a partition-shift DMA
            attT2_odd = persist.tile([64, 512], DT)
            wo2_odd = persist.tile([64, 768], DT)
            ones_sb = persist.tile([65, DH], DT)
            out_partial = persist.tile([128, 8, DIM], ODT)

            # Input DMAs. Transfers serialize on the shared ~360GB/s DMA
            # path, and each dma_start also costs ~625ns on the single
            # HWDGE descriptor generator — so batch the weight loads into
            # one DMA per tensor (per pair-block for w_qk; pair 0 first,
            # it gates the first scores). xT stays per-kt: each tile
            # releases the next accumulation step of the chasing
            # projections. Transfer-queue order = need order:
            # wqk_p0 | xT0..5 | wv | wqk_p1 | wqk_p2 | wo.
            def kpc(dram_ap):
                return dram_ap.rearrange("(k p) c -> p k c", p=128)

            nc.scalar.dma_start(out=wqk[:, :, 0:256], in_=kpc(wqk_d[:, 0:256]))
            for kt in range(6):
                nc.sync.dma_start(out=xT[:, kt, :], in_=xT_d[kt * 128:(kt + 1) * 128, :])
            nc.sync.dma_start(out=wv, in_=kpc(wv_d[:, :]))
            nc.sync.dma_start(out=wqk[:, :, 256:512], in_=kpc(wqk_d[:, 256:512]))
            nc.sync.dma_start(out=wqk[:, :, 512:768], in_=kpc(wqk_d[:, 512:768]))
            nc.sync.dma_start(out=wo, in_=kpc(wo_d[:, :]))
            # ones: v_sb[:, i, h*65 + 64] = 1.0 for all (i, h) (denominator
            # accumulator columns), and a partition-64 row of ones for the
            # denominator broadcast matmul. Constants — memset, no DMA.
            v_ones_view = v_sb.rearrange("p i (h c) -> p i h c", h=6)[:, :, :, 64]
            nc.gpsimd.memset(v_ones_view, 1.0)
            nc.gpsimd.memset(ones_sb[64:65, :], 1.0)
            # pair 2's odd-head w_o rows rebased to partitions 0:64 for the
            # tail's even/odd-split matmuls (Pool; off the startup path)
            nc.gpsimd.tensor_copy(wo2_odd, wo[64:128, 2, :])

            # PE clock warm-up: the tensor engine would otherwise idle
            # through the input-DMA window and start the projections at a
            # low p-state (the cost ramp needs ~3us of continuous busy).
            # Dependency-free 1-wide matmuls (~5ns each) bridge the window.
            warm_c = nc.const_aps.scalar_like(1.0, xT[:, 0, 0:1], dtype=F32)
            warm_ps = psum.tile([1, 1], F32, tag="mm", name="warm_ps")
            for _w in range(WARMUP):
                nc.tensor.matmul(warm_ps, warm_c, warm_c[0:128, 0:1],
                                 start=True, stop=True)

            def qk_pair0():
                """qkT tiles 0,1 (q,k of pair 0) — kt-major across all four
                (mt, chunk) accumulators so each arriving xT tile advances
                everything; after the last tile lands only one kt-step of
                work remains. Uses the two 2-bank score slots (idle until
                the weave starts)."""
                ps = {}
                for mt in (0, 1):
                    ps[mt] = psum.tile([128, 1024], F32, tag="s2",
                                       name=f"qk0_ps_{mt}")
                for kt in range(6):
                    for ch in (0, 1):  # ch0 first: it gates the first scores
                        for mt in (0, 1):
                            nc.tensor.matmul(
                                ps[mt][:, ch * 512:(ch + 1) * 512],
                                wqk[:, kt, mt * 128:(mt + 1) * 128],
                                xT[:, kt, ch * 512:(ch + 1) * 512],
                                start=(kt == 0),
                                stop=(kt == 5),
                            )
                # evict chunk-0 halves first (DVE+ACT in parallel) — the
                # first unit's scores only need them, not chunk 1
                nc.vector.tensor_copy(qkT[:, 0, 0:512], ps[0][:, 0:512])
                nc.scalar.copy(qkT[:, 1, 0:512], ps[1][:, 0:512])
                nc.vector.tensor_copy(qkT[:, 0, 512:1024], ps[0][:, 512:1024])
                nc.scalar.copy(qkT[:, 1, 512:1024], ps[1][:, 512:1024])

            qk_pair0()

            def qk_group(mt, chs=(0, 1)):
                """qkT[mt] = (w_qk col-block mt)^T @ xT, one 512-col chunk
                per call item. Col blocks (pair-packed): mt=2p -> q of pair
                p, 2p+1 -> k."""
                for ch in chs:
                    ps = psum.tile([128, 512], F32, tag="mm", name=f"qk_ps_{mt}_{ch}")
                    for kt in range(6):
                        nc.tensor.matmul(
                            ps,
                            wqk[:, kt, mt * 128:(mt + 1) * 128],
                            xT[:, kt, ch * 512:(ch + 1) * 512],
                            start=(kt == 0),
                            stop=(kt == 5),
                        )
                    nc.vector.tensor_copy(qkT[:, mt, ch * 512:(ch + 1) * 512], ps)

            def v_group(i):
                """v rows-block i = x[i-block] @ w_v, strided into v_sb"""
                ps = psum.tile([128, 384], F32, tag="mm", name=f"v_ps_{i}")
                for kt in range(6):
                    nc.tensor.matmul(
                        ps,
                        xT[:, kt, i * 128:(i + 1) * 128],
                        wv[:, kt, :],
                        start=(kt == 0),
                        stop=(kt == 5),
                    )
                dst = v_sb[:, i, :].rearrange("p (h c) -> p h c", h=6)[:, :, 0:DH]
                src = ps.rearrange("p (h c) -> p h c", h=6)
                nc.vector.tensor_copy(dst, src)

            def attention_unit(p, ch, emit_v, early=(), late=(), last=False):
                """Heads (2p, 2p+1), query chunk ch: both heads' scores into
                one 2-bank PSUM tile -> one 1024-wide exp per key block ->
                AV with fused denominator row -> normalize closures.
                Filler closures fill the tensor engine's exp-latency
                bubbles (the weave is ACT-bound): `early` (the previous
                unit's normalizes — they free the accumulator banks this
                unit's successor waits on) pops at steps 0-1; `late` (out
                rows that read the previous unit's partition-shift DMA,
                ~2.4us after its normalize) drains over steps 3-7."""
                early, late = list(early), list(late)
                qt = 2 * p       # qkT tile of this pair's q
                kt_ = 2 * p + 1  # qkT tile of this pair's k
                o_ps = {}
                for hp in range(2):
                    o_ps[hp] = psum.tile(
                        [65, 512], F32, tag="acc", name=f"oacc_{p}_{ch}_{hp}"
                    )
                for i in range(8):
                    s2 = psum.tile([128, 1024], F32, tag="s2",
                                   name=f"s_{p}_{ch}_{i}")
                    for hp in range(2):
                        lo, hi = hp * 64, hp * 64 + 64
                        nc.tensor.matmul(
                            s2[:, hp * 512:(hp + 1) * 512],
                            qkT[lo:hi, kt_, i * 128:(i + 1) * 128],
                            qkT[lo:hi, qt, ch * 512:(ch + 1) * 512],
                            start=True,
                            stop=True,
                        )
                    pt2 = pt_pool.tile([128, 1024], DT, tag="pt",
                                       name=f"pt_{p}_{ch}_{i}")
                    nc.scalar.activation(
                        pt2, s2, mybir.ActivationFunctionType.Exp, scale=SCALE,
                    )
                    # popped after the scores/exp emission so filler work
                    # never delays the ACT feed in the instruction stream.
                    # `late` spreads over steps 3-7 (mid-unit steps starve
                    # ~190ns/step without filler) but always keeps one item
                    # for step 7, where it lands between the unit's last
                    # scores and its exp-gated final AVs.
                    if early:
                        early.pop(0)()
                    elif i >= 3:
                        while late and len(late) > 7 - i:
                            late.pop(0)()
                        if late and i in (3, 5) and len(late) > (7 - i) // 2:
                            late.pop(0)()
                    if emit_v:
                        # emitted between scores and AV: fills the exp
                        # latency and keeps the w_v DMA off the scores path
                        v_group(i)
                    for hp in range(2):
                        h = 2 * p + hp
                        nc.tensor.matmul(
                            o_ps[hp],
                            v_sb[:, i, h * 65:h * 65 + 65],
                            pt2[:, hp * 512:(hp + 1) * 512],
                            start=(i == 0),
                            stop=(i == 7),
                        )
                # denominator reciprocals start NOW (DVE is off the critical
                # path here) so the accumulator banks free as soon as the
                # deferred bc+mult run in the next unit's filler slots.
                dinvs = {}
                for hp in range(2):
                    dinv = stats.tile(
                        [65, 512], DT, tag=f"dinv{hp}", name=f"dinv_{p}_{ch}_{hp}"
                    )
                    with nc.allow_low_precision(
                        reason="softmax denominators are O(100); rounding "
                        "of 1/denom is in line with the fp16 pipeline"
                    ):
                        nc.vector.reciprocal(dinv[64:65, :], o_ps[hp][64:65, :])
                    dinvs[hp] = dinv

                # normalize: attT rows [hp*64 : hp*64+64] of k-tile p, cols
                # ch. 1/denom is broadcast across partitions with a K=1
                # matmul (ones[1,64]^T @ dinv[1,512] -> [64,512] in PSUM),
                # evicted on Pool (an engine op can read only ONE PSUM
                # operand, and the multiply needs the accumulator). Returned
                # as closures deferred into the next unit's filler stream.
                def make_norm(hp):
                    def go():
                        acc_in = o_ps[hp][0:64, :]
                        bc = psum.tile(
                            [64, 512], F32, tag="mm", name=f"bc_{p}_{ch}_{hp}"
                        )
                        nc.tensor.matmul(
                            bc, ones_sb[64:65, :], dinvs[hp][64:65, :],
                            start=True, stop=True,
                        )
                        bc_sb = stats.tile(
                            [64, 512], DT, tag="bc_sb", name=f"bcs_{p}_{ch}_{hp}"
                        )
                        # bc is PSUM (Pool can't read it): split DVE/ACT so
                        # neither chain serializes both heads
                        if hp == 0:
                            nc.vector.tensor_copy(bc_sb, bc)
                        else:
                            nc.scalar.copy(bc_sb, bc)
                        if hp == 0:
                            nc.vector.tensor_mul(
                                attT[0:64, p, ch * 512:(ch + 1) * 512],
                                acc_in,
                                bc_sb,
                            )
                        elif last:
                            # keep at partitions 0:64; the tail's final
                            # matmuls read it directly (even/odd K-split)
                            nc.vector.tensor_mul(attT2_odd, acc_in, bc_sb)
                        else:
                            tmp = stats.tile(
                                [64, 512], DT, tag="odd_tmp", name=f"ot_{p}_{ch}_{hp}"
                            )
                            nc.vector.tensor_mul(tmp, acc_in, bc_sb)
                            # partition-shifting copy (rows 64:128) is DMA-only
                            nc.sync.dma_start(
                                out=attT[64:128, p, ch * 512:(ch + 1) * 512],
                                in_=tmp,
                            )
                    return go
                return [make_norm(0), make_norm(1)]

            def out_group(i, ch):
                """Half an out-projection row-block: matmuls + copy into the
                per-block staging tile; ch==1 flushes one 768-wide DMA."""
                c0, cw = ((0, 512), (512, 256))[ch]
                if ch == 0:
                    osb = outsb.tile([128, 768], ODT, tag="osb2", name=f"osb2_{i}")
                    _osb_cache[i] = osb
                else:
                    osb = _osb_cache.pop(i)
                ps = psum.tile([128, 512], F32, tag="mm", name=f"o_ps_{i}_{ch}")
                for j in range(3):
                    nc.tensor.matmul(
                        ps[:, 0:cw],
                        attT[:, j, i * 128:(i + 1) * 128],
                        wo[:, j, c0:c0 + cw],
                        start=(j == 0),
                        stop=(j == 2),
                    )
                nc.vector.tensor_copy(osb[:, c0:c0 + cw], ps[:, 0:cw])
                if ch == 1:
                    nc.sync.dma_start(
                        out=out_d[i * 128:(i + 1) * 128, :], in_=osb
                    )

            _osb_cache = {}

            def out_group_partial(i, ch):
                """j=0,1 of (row-block i, chunk ch) into the partial store
                (the finals add the j=2 term on top). Pool eviction: DVE is
                on the normalize chain in the tail."""
                c0, cw = ((0, 512), (512, 256))[ch]
                ps = psum.tile([128, 512], F32, tag="mm", name=f"pp_ps_{i}_{ch}")
                for j in range(2):
                    nc.tensor.matmul(
                        ps[:, 0:cw],
                        attT[:, j, i * 128:(i + 1) * 128],
                        wo[:, j, c0:c0 + cw],
                        start=(j == 0),
                        stop=(j == 1),
                    )
                nc.vector.tensor_copy(out_partial[:, i, c0:c0 + cw], ps[:, 0:cw])

            def out_rows_partial2(i):
                """j=0,1 of row-block i (both column halves) in one freed
                2-bank score slot, evicted with a single 768-wide Pool copy
                (DVE is on the normalize chain in the tail)."""
                ps = psum.tile([128, 1024], F32, tag="s2", name=f"pp_ps_{i}")
                for j in range(2):
                    for c0, cw in ((0, 512), (512, 256)):
                        nc.tensor.matmul(
                            ps[:, c0:c0 + cw],
                            attT[:, j, i * 128:(i + 1) * 128],
                            wo[:, j, c0:c0 + cw],
                            start=(j == 0),
                            stop=(j == 1),
                        )
                nc.scalar.copy(out_partial[:, i, :], ps[:, 0:768])  # ACT: idle tail

            def out_group_final2(i):
                """Row-block i of the tail: j=2 contribution split into the
                even head (attT partitions 0:64) and the odd head
                (attT2_odd, partitions 0:64 — no partition-shift DMA wait),
                added onto the precomputed j=0,1 partials, flushed per
                512/256 half so the last DMA is small and early. Uses the
                accumulator banks (freed by the last unit's normalizes);
                adds alternate DVE/Pool."""
                qs = (i - 4) * 128
                ptag = "acc" if i % 2 == 0 else "mm"
                osb = outsb.tile([128, 768], ODT, tag="osb2", name=f"osb2_{i}")
                for c0, cw in ((0, 512), (512, 256)):
                    ps = psum.tile([128, 512], F32, tag=ptag, name=f"f2_ps_{i}_{c0}")
                    nc.tensor.matmul(
                        ps[:, 0:cw],
                        attT[0:64, 2, i * 128:(i + 1) * 128],
                        wo[0:64, 2, c0:c0 + cw],
                        start=True, stop=False,
                    )
                    nc.tensor.matmul(
                        ps[:, 0:cw],
                        attT2_odd[:, qs:qs + 128],
                        wo2_odd[:, c0:c0 + cw],
                        start=False, stop=True,
                    )
                    nc.vector.tensor_add(
                        osb[:, c0:c0 + cw], ps[:, 0:cw],
                        out_partial[:, i, c0:c0 + cw],
                    )
                    nc.sync.dma_start(
                        out=out_d[i * 128:(i + 1) * 128, c0:c0 + cw],
                        in_=osb[:, c0:c0 + cw],
                    )

            # The weave. Query-chunk-0 units first: once (0,0),(1,0),(2,0)
            # are done, output row-blocks 0..3 are fully determined, so the
            # out-projection for rows 0-3 (and its DMA) spreads across the
            # chunk-1 units. Filler loads are balanced against each unit's
            # ~8.3us of exp work (u0 carries the v projection inherently).
            # qkT tiles feed the NEXT unit: pair 1 (tiles 2,3) from u0's
            # fillers, pair 2 (tiles 4,5) from u1's; chunk-1 q columns
            # (consumed only by the chunk-1 units) trail in u1/u2.
            nrm = attention_unit(0, 0, emit_v=True, late=[
                lambda: qk_group(3, chs=(0,)),
                lambda: qk_group(2, chs=(0,)),
                lambda: qk_group(3, chs=(1,)),
            ])
            nrm = attention_unit(1, 0, emit_v=False, early=nrm, late=[
                lambda: qk_group(5, chs=(0,)),
                lambda: qk_group(4, chs=(0,)),
                lambda: qk_group(5, chs=(1,)),
            ])
            nrm = attention_unit(2, 0, emit_v=False, early=nrm, late=[
                lambda: qk_group(2, chs=(1,)),
                lambda: qk_group(4, chs=(1,)),
            ])
            nrm = attention_unit(0, 1, emit_v=False, early=nrm, late=[
                lambda i=i, ch=c: out_group(i, ch)
                for (i, c) in ((0, 0), (0, 1), (1, 0))
            ])
            nrm = attention_unit(1, 1, emit_v=False, early=nrm, late=[
                lambda i=i, ch=c: out_group(i, ch)
                for (i, c) in ((1, 1), (2, 0), (2, 1))
            ])
            nrm = attention_unit(2, 1, emit_v=False, early=nrm, late=[
                lambda i=i, ch=c: out_group(i, ch)
                for (i, c) in ((3, 0), (3, 1))
            ] + [
                lambda i=i, ch=c: out_group_partial(i, ch)
                for i in (4, 5) for c in (0, 1)
            ], last=True)
            # Pipelined tail: rows 4,5's j=0,1 partials ran as the last
            # unit's late fillers (pairs 0,1 chunk 1 normalized by then);
            # rows 6,7's fill the normalize-chain latency here; finals then
            # stream one half-row DMA at a time.
            nrm[0]()
            out_rows_partial2(6)
            out_rows_partial2(7)
            nrm[1]()
            for i in range(4, 8):
                out_group_final2(i)

    nc.compile()
    return nc


_NC_CACHE = {}


def _get_nc():
    if MODE not in _NC_CACHE:
        _NC_CACHE[MODE] = build_nc(MODE)
    return _NC_CACHE[MODE]


def kernel(x, w_qkv, w_out, b_out):
    x = np.asarray(x, dtype=np.float32)
    w_qkv = np.asarray(w_qkv, dtype=np.float32)
    w_out = np.asarray(w_out, dtype=np.float32)
    b_out = np.asarray(b_out, dtype=np.float32)

    nc = _get_nc()
    if MODE == "f16":
        x = x.astype(np.float16)
        w_qkv = w_qkv.astype(np.float16)
        w_out = w_out.astype(np.float16)
    in_maps = []
    for c in range(NCORES):
        b = c // 2
        hs = (c % 2) * HEADS_PER_CORE
        q_cols = w_qkv[:, hs * DH:(hs + 6) * DH]
        k_cols = w_qkv[:, 768 + hs * DH:768 + (hs + 6) * DH]
        # pair-packed: [q_p0 | k_p0 | q_p1 | k_p1 | q_p2 | k_p2], 128 each
        wqk_packed = np.concatenate(
            [blk for p in range(3)
             for blk in (q_cols[:, p * 128:(p + 1) * 128],
                         k_cols[:, p * 128:(p + 1) * 128])],
            axis=1,
        )
        in_maps.append({
            "xT": np.ascontiguousarray(x[b].T),
            "w_qk": np.ascontiguousarray(wqk_packed),
            "w_v": np.ascontiguousarray(w_qkv[:, 1536 + hs * DH:1536 + (hs + 6) * DH]),
            "w_o": np.ascontiguousarray(w_out[hs * DH:(hs + 6) * DH, :]),
        })

    res = run_bass_kernel_spmd(nc, in_maps, core_ids=list(range(NCORES))).results

    out = np.empty((4, N, DIM), dtype=np.float32)
    for b in range(4):
        out[b] = (res[2 * b]["out"].astype(np.float32)
                  + res[2 * b + 1]["out"].astype(np.float32) + b_out)
    return out


# revision 42
# speedup vs baseline: 1.0393x; 1.0100x over previous
"""Multi-head attention forward (B=4, N=1024, D=768, H=12, dh=64) on 8 TRN2 cores.

Sharding: (batch, head-group) — core c handles batch b = c//2 and heads
hs..hs+5 where hs = (c%2)*6.  Each core computes its 6 heads' contribution
to out[b] = attn(x[b]) @ W_out_rows(for its heads); host sums the two
partials per batch and adds the bias (the "all-reduce after final linear").

Per-core dataflow (all contraction dims on SBUF partitions), fp16 wire
dtype (host casts; fp32 PSUM accumulation; ~1e-3 end-to-end rel err):
  qkT  [768,1024] = w_qk^T @ x^T          (d-major q,k — feeds scores;
                                           w_qk cols pair-packed
                                           [q_p0|k_p0|q_p1|k_p1|q_p2|k_p2])
  v    [1024,390] = x @ w_v (+ ones col)  (n-major v — feeds AV^T)
  S^T  [1024,1024]/head = k_h @ q_h^T     (keys on partitions, 2 heads
                                           row-packed in the PE array,
                                           both heads' scores in one
                                           2-bank PSUM tile so each
                                           (i, pair) step is ONE 1024-wide
                                           exp — ACT is the weave pacer)
  P^T  = exp(S^T * scale)                 (no max-sub: scores ~ N(0,1))
  oT   [65,1024]/head = [v_h|1]^T @ P^T   (row 64 = softmax denominators)
  attT = oT[0:64] * (1/denom)             (K=1 matmul broadcasts 1/denom
                                           across partitions, DVE mult)
  out  [1024,768] = attT^T @ w_o          (partial; host all-reduce)

The weave: filler work (projections for later pairs, v blocks, finished
output rows) is interleaved into each attention unit's emission stream so
the tensor engine's exp-latency bubbles are filled, balanced against the
unit's ~8.3us of ACT exp work. The last unit ends with a pipelined tail:
row partials first, then the normalize chain split even/odd so the final
j=2 matmuls read the odd-head tile directly (no SBUF->SBUF partition-shift
DMA on the critical path), one output DMA per finished half-row.
"""
import os
import sys

sys.path.insert(0, "/opt/trn_rl_repo")

# The kernel needs the axon-tunneled TRN2 PJRT backend; a JAX_PLATFORMS=cpu
# pin (common for reference-side jax) would hide the NeuronCores.
if os.environ.get("JAX_PLATFORMS", "").strip() == "cpu":
    del os.environ["JAX_PLATFORMS"]

import numpy as np
import concourse.bass as bass
import concourse.bacc as bacc
import concourse.tile as tile
from concourse import mybir
from concourse.bass_utils import run_bass_kernel_spmd
from contextlib import ExitStack

F32 = mybir.dt.float32
F32R = mybir.dt.float32r
F16 = mybir.dt.float16

DIM = 768
N = 1024
HEADS_PER_CORE = 6
DH = 64
SCALE = DH ** -0.5
NCORES = 8

# "f16"  = fp16 pipeline (1 cyc/row PE at any width, half DMA traffic;
#          ~1e-3 end-to-end rel err)
# "f32r" = TF32-like matmul mode (~4e-4 end-to-end rel err)
MODE = os.environ.get("ATTN_MM_DTYPE", "f16")

# PE p-state warm-up reps: dependency-free 1-wide matmuls bridging the
# input-DMA window so real matmuls start at full clock.
WARMUP = int(os.environ.get("ATTN_WARMUP", "1300"))


def build_nc(mode=MODE):
    DT = {"f32r": F32R, "f32": F32, "f16": F16}[mode]
    ODT = F16 if mode == "f16" else F32
    nc = bacc.Bacc("TRN2", target_bir_lowering=False, debug=False)

    xT_d = nc.declare_dram_parameter("xT", [DIM, N], DT, isOutput=False)
    wqk_d = nc.declare_dram_parameter("w_qk", [DIM, 768], DT, isOutput=False)
    wv_d = nc.declare_dram_parameter("w_v", [DIM, 384], DT, isOutput=False)
    wo_d = nc.declare_dram_parameter("w_o", [384, DIM], DT, isOutput=False)
    out_d = nc.declare_dram_parameter("out", [N, DIM], ODT, isOutput=True)

    with tile.TileContext(nc) as tc:
        with ExitStack() as ctx:
            persist = ctx.enter_context(tc.tile_pool(name="persist", bufs=1))
            pt_pool = ctx.enter_context(tc.tile_pool(name="pt", bufs=6))
            stats = ctx.enter_context(tc.tile_pool(name="stats", bufs=3))
            outsb = ctx.enter_context(tc.tile_pool(name="outsb", bufs=4))
            # One PSUM pool, tag-sized: "s2" 2x[128,1024] (4 banks, score
            # double-tiles), "acc" 2x[65,512] (2, AV accumulators), "mm"
            # 2x[128,512] (2, everything else). 8 banks total.
            psum = ctx.enter_context(tc.tile_pool(name="psum", bufs=2, space="PSUM"))

            xT = persist.tile([128, 6, N], DT)
            wqk = persist.tile([128, 6, 768], DT)
            wv = persist.tile([128, 6, 384], DT)
            wo = persist.tile([128, 3, 768], DT)
            qkT = persist.tile([128, 6, N], DT)
            v_sb = persist.tile([128, 8, 6 * 65], DT)
            attT = persist.tile([128, 3, N], DT)
            # last unit's odd-head normalized rows, kept at partitions 0:64
            # so the final j=2 matmuls read them without a partition-shift DMA
            attT2_odd = persist.tile([64, 512], DT)
            wo2_odd = persist.tile([64, 768], DT)
            ones_sb = persist.tile([65, DH], DT)
            out_partial = persist.tile([128, 8, DIM], ODT)

            # Input DMAs. Transfers serialize on the shared ~360GB/s DMA
            # path, and each dma_start also costs ~625ns on the single
            # HWDGE descriptor generator — so batch the weight loads into
            # one DMA per tensor (per pair-block for w_qk; pair 0 first,
            # it gates the first scores). xT stays per-kt: each tile
            # releases the next accumulation step of the chasing
            # projections. Transfer-queue order = need order:
            # wqk_p0 | xT0..5 | wv | wqk_p1 | wqk_p2 | wo.
            def kpc(dram_ap):
                return dram_ap.rearrange("(k p) c -> p k c", p=128)

            nc.scalar.dma_start(out=wqk[:, :, 0:256], in_=kpc(wqk_d[:, 0:256]))
            for kt in range(6):
                nc.sync.dma_start(out=xT[:, kt, :], in_=xT_d[kt * 128:(kt + 1) * 128, :])
            nc.sync.dma_start(out=wv, in_=kpc(wv_d[:, :]))
            nc.sync.dma_start(out=wqk[:, :, 256:512], in_=kpc(wqk_d[:, 256:512]))
            nc.sync.dma_start(out=wqk[:, :, 512:768], in_=kpc(wqk_d[:, 512:768]))
            nc.sync.dma_start(out=wo, in_=kpc(wo_d[:, :]))
            # ones: v_sb[:, i, h*65 + 64] = 1.0 for all (i, h) (denominator
            # accumulator columns), and a partition-64 row of ones for the
            # denominator broadcast matmul. Constants — memset, no DMA.
            v_ones_view = v_sb.rearrange("p i (h c) -> p i h c", h=6)[:, :, :, 64]
            nc.gpsimd.memset(v_ones_view, 1.0)
            nc.gpsimd.memset(ones_sb[64:65, :], 1.0)
            # pair 2's odd-head w_o rows rebased to partitions 0:64 for the
            # tail's even/odd-split matmuls (Pool; off the startup path)
            nc.gpsimd.tensor_copy(wo2_odd, wo[64:128, 2, :])

            # PE clock warm-up: the tensor engine would otherwise idle
            # through the input-DMA window and start the projections at a
            # low p-state (the cost ramp needs ~3us of continuous busy).
            # Dependency-free 1-wide matmuls (~5ns each) bridge the window.
            warm_c = nc.const_aps.scalar_like(1.0, xT[:, 0, 0:1], dtype=F32)
            warm_ps = psum.tile([1, 1], F32, tag="mm", name="warm_ps")
            for _w in range(WARMUP):
                nc.tensor.matmul(warm_ps, warm_c, warm_c[0:128, 0:1],
                                 start=True, stop=True)

            def qk_pair0():
                """qkT tiles 0,1 (q,k of pair 0) — kt-major across all four
                (mt, chunk) accumulators so each arriving xT tile advances
                everything; after the last tile lands only one kt-step of
                work remains. Uses the two 2-bank score slots (idle until
                the weave starts)."""
                ps = {}
                for mt in (0, 1):
                    ps[mt] = psum.tile([128, 1024], F32, tag="s2",
                                       name=f"qk0_ps_{mt}")
                for kt in range(6):
                    for ch in (0, 1):  # ch0 first: it gates the first scores
                        for mt in (0, 1):
                            nc.tensor.matmul(
                                ps[mt][:, ch * 512:(ch + 1) * 512],
                                wqk[:, kt, mt * 128:(mt + 1) * 128],
                                xT[:, kt, ch * 512:(ch + 1) * 512],
                                start=(kt == 0),
                                stop=(kt == 5),
                            )
                # evict chunk-0 halves first (DVE+ACT in parallel) — the
                # first unit's scores only need them, not chunk 1
                nc.vector.tensor_copy(qkT[:, 0, 0:512], ps[0][:, 0:512])
                nc.scalar.copy(qkT[:, 1, 0:512], ps[1][:, 0:512])
                nc.vector.tensor_copy(qkT[:, 0, 512:1024], ps[0][:, 512:1024])
                nc.scalar.copy(qkT[:, 1, 512:1024], ps[1][:, 512:1024])

            qk_pair0()

            def qk_group(mt, chs=(0, 1)):
                """qkT[mt] = (w_qk col-block mt)^T @ xT, one 512-col chunk
                per call item. Col blocks (pair-packed): mt=2p -> q of pair
                p, 2p+1 -> k."""
                for ch in chs:
                    ps = psum.tile([128, 512], F32, tag="mm", name=f"qk_ps_{mt}_{ch}")
                    for kt in range(6):
                        nc.tensor.matmul(
                            ps,
                            wqk[:, kt, mt * 128:(mt + 1) * 128],
                            xT[:, kt, ch * 512:(ch + 1) * 512],
                            start=(kt == 0),
                            stop=(kt == 5),
                        )
                    nc.vector.tensor_copy(qkT[:, mt, ch * 512:(ch + 1) * 512], ps)

            def v_group(i):
                """v rows-block i = x[i-block] @ w_v, strided into v_sb"""
                ps = psum.tile([128, 384], F32, tag="mm", name=f"v_ps_{i}")
                for kt in range(6):
                    nc.tensor.matmul(
                        ps,
                        xT[:, kt, i * 128:(i + 1) * 128],
                        wv[:, kt, :],
                        start=(kt == 0),
                        stop=(kt == 5),
                    )
                dst = v_sb[:, i, :].rearrange("p (h c) -> p h c", h=6)[:, :, 0:DH]
                src = ps.rearrange("p (h c) -> p h c", h=6)
                nc.vector.tensor_copy(dst, src)

            def attention_unit(p, ch, emit_v, early=(), late=(), last=False):
                """Heads (2p, 2p+1), query chunk ch: both heads' scores into
                one 2-bank PSUM tile -> one 1024-wide exp per key block ->
                AV with fused denominator row -> normalize closures.
                Filler closures fill the tensor engine's exp-latency
                bubbles (the weave is ACT-bound): `early` (the previous
                unit's normalizes — they free the accumulator banks this
                unit's successor waits on) pops at steps 0-1; `late` (out
                rows that read the previous unit's partition-shift DMA,
                ~2.4us after its normalize) drains over steps 3-7."""
                early, late = list(early), list(late)
                qt = 2 * p       # qkT tile of this pair's q
                kt_ = 2 * p + 1  # qkT tile of this pair's k
                o_ps = {}
                for hp in range(2):
                    o_ps[hp] = psum.tile(
                        [65, 512], F32, tag="acc", name=f"oacc_{p}_{ch}_{hp}"
                    )
                for i in range(8):
                    s2 = psum.tile([128, 1024], F32, tag="s2",
                                   name=f"s_{p}_{ch}_{i}")
                    for hp in range(2):
                        lo, hi = hp * 64, hp * 64 + 64
                        nc.tensor.matmul(
                            s2[:, hp * 512:(hp + 1) * 512],
                            qkT[lo:hi, kt_, i * 128:(i + 1) * 128],
                            qkT[lo:hi, qt, ch * 512:(ch + 1) * 512],
                            start=True,
                            stop=True,
                        )
                    pt2 = pt_pool.tile([128, 1024], DT, tag="pt",
                                       name=f"pt_{p}_{ch}_{i}")
                    nc.scalar.activation(
                        pt2, s2, mybir.ActivationFunctionType.Exp, scale=SCALE,
                    )
                    # popped after the scores/exp emission so filler work
                    # never delays the ACT feed in the instruction stream.
                    # `late` spreads over steps 3-7 (mid-unit steps starve
                    # ~190ns/step without filler) but always keeps one item
                    # for step 7, where it lands between the unit's last
                    # scores and its exp-gated final AVs.
                    if early:
                        early.pop(0)()
                    elif i >= 3:
                        while late and len(late) > 7 - i:
                            late.pop(0)()
                        if late and i in (3, 5) and len(late) > (7 - i) // 2:
                            late.pop(0)()
                    if emit_v:
                        # emitted between scores and AV: fills the exp
                        # latency and keeps the w_v DMA off the scores path
                        v_group(i)
                    for hp in range(2):
                        h = 2 * p + hp
                        nc.tensor.matmul(
                            o_ps[hp],
                            v_sb[:, i, h * 65:h * 65 + 65],
                            pt2[:, hp * 512:(hp + 1) * 512],
                            start=(i == 0),
                            stop=(i == 7),
                        )
                # denominator reciprocals start NOW (DVE is off the critical
                # path here) so the accumulator banks free as soon as the
                # deferred bc+mult run in the next unit's filler slots.
                dinvs = {}
                for hp in range(2):
                    dinv = stats.tile(
                        [65, 512], DT, tag=f"dinv{hp}", name=f"dinv_{p}_{ch}_{hp}"
                    )
                    with nc.allow_low_precision(
                        reason="softmax denominators are O(100); rounding "
                        "of 1/denom is in line with the fp16 pipeline"
                    ):
                        nc.vector.reciprocal(dinv[64:65, :], o_ps[hp][64:65, :])
                    dinvs[hp] = dinv

                # normalize: attT rows [hp*64 : hp*64+64] of k-tile p, cols
                # ch. 1/denom is broadcast across partitions with a K=1
                # matmul (ones[1,64]^T @ dinv[1,512] -> [64,512] in PSUM),
                # evicted on Pool (an engine op can read only ONE PSUM
                # operand, and the multiply needs the accumulator). Returned
                # as closures deferred into the next unit's filler stream.
                def make_norm(hp):
                    def go():
                        acc_in = o_ps[hp][0:64, :]
                        bc = psum.tile(
                            [64, 512], F32, tag="mm", name=f"bc_{p}_{ch}_{hp}"
                        )
                        nc.tensor.matmul(
                            bc, ones_sb[64:65, :], dinvs[hp][64:65, :],
                            start=True, stop=True,
                        )
                        bc_sb = stats.tile(
                            [64, 512], DT, tag="bc_sb", name=f"bcs_{p}_{ch}_{hp}"
                        )
                        # bc is PSUM (Pool can't read it): split DVE/ACT so
                        # neither chain serializes both heads
                        if hp == 0:
                            nc.vector.tensor_copy(bc_sb, bc)
                        else:
                            nc.scalar.copy(bc_sb, bc)
                        if hp == 0:
                            nc.vector.tensor_mul(
                                attT[0:64, p, ch * 512:(ch + 1) * 512],
                                acc_in,
                                bc_sb,
                            )
                        elif last:
                            # keep at partitions 0:64; the tail's final
                            # matmuls read it directly (even/odd K-split)
                            nc.vector.tensor_mul(attT2_odd, acc_in, bc_sb)
                        else:
                            tmp = stats.tile(
                                [64, 512], DT, tag="odd_tmp", name=f"ot_{p}_{ch}_{hp}"
                            )
                            nc.vector.tensor_mul(tmp, acc_in, bc_sb)
                            # partition-shifting copy (rows 64:128) is DMA-only
                            nc.sync.dma_start(
                                out=attT[64:128, p, ch * 512:(ch + 1) * 512],
                                in_=tmp,
                            )
                    return go
                return [make_norm(0), make_norm(1)]

            def out_group(i, ch):
                """Half an out-projection row-block: matmuls + copy into the
                per-block staging tile; ch==1 flushes one 768-wide DMA."""
                c0, cw = ((0, 512), (512, 256))[ch]
                if ch == 0:
                    osb = outsb.tile([128, 768], ODT, tag="osb2", name=f"osb2_{i}")
                    _osb_cache[i] = osb
                else:
                    osb = _osb_cache.pop(i)
                ps = psum.tile([128, 512], F32, tag="mm", name=f"o_ps_{i}_{ch}")
                for j in range(3):
                    nc.tensor.matmul(
                        ps[:, 0:cw],
                        attT[:, j, i * 128:(i + 1) * 128],
                        wo[:, j, c0:c0 + cw],
                        start=(j == 0),
                        stop=(j == 2),
                    )
                nc.vector.tensor_copy(osb[:, c0:c0 + cw], ps[:, 0:cw])
                if ch == 1:
                    nc.sync.dma_start(
                        out=out_d[i * 128:(i + 1) * 128, :], in_=osb
                    )

            _osb_cache = {}

            def out_group_partial(i, ch):
                """j=0,1 of (row-block i, chunk ch) into the partial store
                (the finals add the j=2 term on top). Pool eviction: DVE is
                on the normalize chain in the tail."""
                c0, cw = ((0, 512), (512, 256))[ch]
                ps = psum.tile([128, 512], F32, tag="mm", name=f"pp_ps_{i}_{ch}")
                for j in range(2):
                    nc.tensor.matmul(
                        ps[:, 0:cw],
                        attT[:, j, i * 128:(i + 1) * 128],
                        wo[:, j, c0:c0 + cw],
                        start=(j == 0),
                        stop=(j == 1),
                    )
                nc.vector.tensor_copy(out_partial[:, i, c0:c0 + cw], ps[:, 0:cw])

            def out_rows_fused_start(i):
                """j=0,1 of row-block i (both column halves) into a freed
                2-bank score slot, accumulation left OPEN — the j=2 finish
                lands on top (no separate partial-store + add round trip)."""
                ps = psum.tile([128, 1024], F32, tag="s2", name=f"pp_ps_{i}")
                for j in range(2):
                    for c0, cw in ((0, 512), (512, 256)):
                        nc.tensor.matmul(
                            ps[:, c0:c0 + cw],
                            attT[:, j, i * 128:(i + 1) * 128],
                            wo[:, j, c0:c0 + cw],
                            start=(j == 0),
                            stop=False,
                        )
                return ps

            def out_rows_fused_finish(i, ps):
                """j=2 (even/odd split) accumulated onto the open j=0,1
                sums; one 768-wide ACT eviction (idle at the tail) and one
                DMA per row."""
                qs = (i - 4) * 128
                for c0, cw in ((0, 512), (512, 256)):
                    nc.tensor.matmul(
                        ps[:, c0:c0 + cw],
                        attT[0:64, 2, i * 128:(i + 1) * 128],
                        wo[0:64, 2, c0:c0 + cw],
                        start=False, stop=False,
                    )
                    nc.tensor.matmul(
                        ps[:, c0:c0 + cw],
                        attT2_odd[:, qs:qs + 128],
                        wo2_odd[:, c0:c0 + cw],
                        start=False, stop=True,
                    )
                osb = outsb.tile([128, 768], ODT, tag="osb2", name=f"osb2_{i}")
                nc.scalar.copy(osb, ps[:, 0:768])
                nc.sync.dma_start(out=out_d[i * 128:(i + 1) * 128, :], in_=osb)

            def out_group_final2(i):
                """Row-block i of the tail: j=2 contribution split into the
                even head (attT partitions 0:64) and the odd head
                (attT2_odd, partitions 0:64 — no partition-shift DMA wait),
                added onto the precomputed j=0,1 partials, flushed per
                512/256 half so the last DMA is small and early. Uses the
                accumulator banks (freed by the last unit's normalizes);
                adds alternate DVE/Pool."""
                qs = (i - 4) * 128
                ptag = "acc" if i % 2 == 0 else "mm"
                osb = outsb.tile([128, 768], ODT, tag="osb2", name=f"osb2_{i}")
                for c0, cw in ((0, 512), (512, 256)):
                    ps = psum.tile([128, 512], F32, tag=ptag, name=f"f2_ps_{i}_{c0}")
                    nc.tensor.matmul(
                        ps[:, 0:cw],
                        attT[0:64, 2, i * 128:(i + 1) * 128],
                        wo[0:64, 2, c0:c0 + cw],
                        start=True, stop=False,
                    )
                    nc.tensor.matmul(
                        ps[:, 0:cw],
                        attT2_odd[:, qs:qs + 128],
                        wo2_odd[:, c0:c0 + cw],
                        start=False, stop=True,
                    )
                    nc.vector.tensor_add(
                        osb[:, c0:c0 + cw], ps[:, 0:cw],
                        out_partial[:, i, c0:c0 + cw],
                    )
                    nc.sync.dma_start(
                        out=out_d[i * 128:(i + 1) * 128, c0:c0 + cw],
                        in_=osb[:, c0:c0 + cw],
                    )

            # The weave. Query-chunk-0 units first: once (0,0),(1,0),(2,0)
            # are done, output row-blocks 0..3 are fully determined, so the
            # out-projection for rows 0-3 (and its DMA) spreads across the
            # chunk-1 units. Filler loads are balanced against each unit's
            # ~8.3us of exp work (u0 carries the v projection inherently).
            # qkT tiles feed the NEXT unit: pair 1 (tiles 2,3) from u0's
            # fillers, pair 2 (tiles 4,5) from u1's; chunk-1 q columns
            # (consumed only by the chunk-1 units) trail in u1/u2.
            nrm = attention_unit(0, 0, emit_v=True, late=[
                lambda: qk_group(3, chs=(0,)),
                lambda: qk_group(2, chs=(0,)),
                lambda: qk_group(3, chs=(1,)),
            ])
            nrm = attention_unit(1, 0, emit_v=False, early=nrm, late=[
                lambda: qk_group(5, chs=(0,)),
                lambda: qk_group(4, chs=(0,)),
                lambda: qk_group(5, chs=(1,)),
            ])
            nrm = attention_unit(2, 0, emit_v=False, early=nrm, late=[
                lambda: qk_group(2, chs=(1,)),
                lambda: qk_group(4, chs=(1,)),
            ])
            nrm = attention_unit(0, 1, emit_v=False, early=nrm, late=[
                lambda i=i, ch=c: out_group(i, ch)
                for (i, c) in ((0, 0), (0, 1), (1, 0))
            ])
            nrm = attention_unit(1, 1, emit_v=False, early=nrm, late=[
                lambda i=i, ch=c: out_group(i, ch)
                for (i, c) in ((1, 1), (2, 0), (2, 1))
            ])
            nrm = attention_unit(2, 1, emit_v=False, early=nrm, late=[
                lambda i=i, ch=c: out_group(i, ch)
                for (i, c) in ((3, 0), (3, 1))
            ] + [
                lambda i=i, ch=c: out_group_partial(i, ch)
                for i in (4, 5) for c in (0, 1)
            ], last=True)
            # Pipelined tail: rows 4,5's j=0,1 partials ran as the last
            # unit's late fillers (pairs 0,1 chunk 1 normalized by then);
            # rows 6,7's open-accumulation partials fill the normalize
            # chain's latency; then rows 4,5 finish via the add path (DVE)
            # while rows 6,7 finish in PSUM and evict on idle ACT.
            ps6 = out_rows_fused_start(6)
            ps7 = out_rows_fused_start(7)
            nrm[0]()
            nrm[1]()
            out_group_final2(4)
            out_group_final2(5)
            out_rows_fused_finish(6, ps6)
            out_rows_fused_finish(7, ps7)

    nc.compile()
    return nc


_NC_CACHE = {}


def _get_nc():
    if MODE not in _NC_CACHE:
        _NC_CACHE[MODE] = build_nc(MODE)
    return _NC_CACHE[MODE]


def kernel(x, w_qkv, w_out, b_out):
    x = np.asarray(x, dtype=np.float32)
    w_qkv = np.asarray(w_qkv, dtype=np.float32)
    w_out = np.asarray(w_out, dtype=np.float32)
    b_out = np.asarray(b_out, dtype=np.float32)

    nc = _get_nc()
    if MODE == "f16":
        x = x.astype(np.float16)
        w_qkv = w_qkv.astype(np.float16)
        w_out = w_out.astype(np.float16)
    in_maps = []
    for c in range(NCORES):
        b = c // 2
        hs = (c % 2) * HEADS_PER_CORE
        q_cols = w_qkv[:, hs * DH:(hs + 6) * DH]
        k_cols = w_qkv[:, 768 + hs * DH:768 + (hs + 6) * DH]
        # pair-packed: [q_p0 | k_p0 | q_p1 | k_p1 | q_p2 | k_p2], 128 each
        wqk_packed = np.concatenate(
            [blk for p in range(3)
             for blk in (q_cols[:, p * 128:(p + 1) * 128],
                         k_cols[:, p * 128:(p + 1) * 128])],
            axis=1,
        )
        in_maps.append({
            "xT": np.ascontiguousarray(x[b].T),
            "w_qk": np.ascontiguousarray(wqk_packed),
            "w_v": np.ascontiguousarray(w_qkv[:, 1536 + hs * DH:1536 + (hs + 6) * DH]),
            "w_o": np.ascontiguousarray(w_out[hs * DH:(hs + 6) * DH, :]),
        })

    res = run_bass_kernel_spmd(nc, in_maps, core_ids=list(range(NCORES))).results

    out = np.empty((4, N, DIM), dtype=np.float32)
    for b in range(4):
        out[b] = (res[2 * b]["out"].astype(np.float32)
                  + res[2 * b + 1]["out"].astype(np.float32) + b_out)
    return out
